# revision 1
# baseline (speedup 1.0000x reference)
"""GPTBigCode transformer block (MQA) on 8 trn2 NeuronCores.

Sharding: data-parallel over batch (4) x sequence-split (2) per batch
element. Core c handles batch c//2 and the interleaved token blocks
{2i + c%2 : i in 0..7} (128 tokens each), which balances causal-attention
work across cores and needs NO collectives: K/V (single MQA head) are
recomputed per core from the full per-batch hidden states.

In-kernel dataflow keeps activations feature-on-partition ("T layout")
so matmul chains need no operand transposes; activations are transposed
once per stage on the tensor engine. LN gains/biases are folded into the
following matmul weights on the host; matmul inputs are bf16, all
accumulation/softmax/residual math is f32.
"""

import numpy as np
import ml_dtypes

# ---------------------------------------------------------------------------
# Workaround: this container's walrus build rejects >1 sync-wait on
# CTRL-class (Drain) instructions. Split the Tile tail-drain's waits into
# individual wait-carrying NOPs on the SP engine.
import bass_rust
from concourse.tile import TileContext
from concourse.vector_clock import ScopedClock


def _patched_drain_and_barrier(self, tick_clock, wait_clock):
    nc = self.nc
    drain_inst = nc.sync.drain()
    wait_clock.add_sem_waits(
        drain_inst.ins, ScopedClock({None: tick_clock.global_clock})
    )
    si = drain_inst.ins.sync_info
    waits = list(si.on_wait) if si and si.on_wait else []
    if len(waits) > 1:
        drain_inst.ins.sync_info = bass_rust.SyncInfo(
            on_wait=waits[:1],
            on_update=list(si.on_update) if si.on_update else [],
        )
        for w in waits[1:]:
            n = nc.sync.nop(nofuse=True, hint="split_drain_wait")
            n.ins.sync_info = bass_rust.SyncInfo(on_wait=[w], on_update=[])
    nc.all_engine_barrier()
    assert self.sems is not None
    popped = nc._tile_sem_poison_stack.pop()
    assert popped is self._sem_poison
    nc.clear_and_free_semaphores(list(self.sems.allocated().values()))
    nc.all_engine_barrier()


TileContext._drain_and_barrier = _patched_drain_and_barrier


def _split_excess_waits(nc, max_waits=1):
    """Rewrite every instruction carrying more than `max_waits` sem-waits:
    excess waits move onto same-engine NOPs inserted just before it."""
    all_bbs = [bb for fn in nc.m.functions for bb in fn.blocks]
    for bb in all_bbs:
        insts = list(bb.instructions)
        new_list = []
        changed = False
        for inst in insts:
            si = inst.sync_info
            waits = list(si.on_wait) if si and si.on_wait else []
            if len(waits) > max_waits:
                changed = True
                inst.sync_info = bass_rust.SyncInfo(
                    on_wait=waits[:max_waits],
                    on_update=list(si.on_update) if si.on_update else [],
                )
                for w in waits[max_waits:]:
                    nop_bi = nc.engines[inst.engine].nop(
                        nofuse=True, hint="wsplit"
                    )
                    nop = nop_bi.ins
                    cur = nc.cur_bb.bb
                    cl = list(cur.instructions)
                    assert cl and cl[-1].name == nop.name, "nop not appended last"
                    cur.instructions = cl[:-1]
                    nop.sync_info = bass_rust.SyncInfo(on_wait=[w], on_update=[])
                    new_list.append(nop)
            new_list.append(inst)
        if changed:
            bb.instructions = new_list
# ---------------------------------------------------------------------------

import concourse.bass as bass
import concourse.mybir as mybir
from concourse.bass_utils import run_bass_kernel_spmd
from concourse.masks import make_identity

f32 = mybir.dt.float32
bf16 = mybir.dt.bfloat16
AF = mybir.ActivationFunctionType
ALU = mybir.AluOpType

H = 2048
NH = 16
D = 128
INTER = 8192
S = 2048
B = 4
NQ = 1024          # query tokens per core
QT = NQ // 128     # 8 local q tiles
KT = S // 128      # 16 key tiles
HT = H // 128      # 16 hidden tiles
IT = INTER // 128  # 64
EPS = 1e-5
NEG = -1e30


def _layernorm(nc, pool, x_t, ln_out, eps_t):
    """x_t [128, H] f32 -> ln_out [128, H] (normalized, no gain/bias)."""
    st = pool.tile([128, 4, 6], f32, tag="st", bufs=3, name="st")
    xr = x_t.rearrange("p (g f) -> p g f", g=4)
    for g in range(4):
        nc.vector.bn_stats(out=st[:, g, :], in_=xr[:, g, :])
    mv = pool.tile([128, 2], f32, tag="mv", bufs=3, name="mv")
    nc.vector.bn_aggr(out=mv, in_=st)
    rstd = pool.tile([128, 1], f32, tag="rstd", bufs=3, name="rstd")
    nc.scalar.activation(out=rstd, in_=mv[:, 1:2], func=AF.Sqrt, bias=eps_t)
    nc.vector.reciprocal(out=rstd, in_=rstd)
    nc.vector.tensor_scalar(
        out=ln_out, in0=x_t, scalar1=mv[:, 0:1], scalar2=rstd,
        op0=ALU.subtract, op1=ALU.mult,
    )


def _build_program():
    nc = bass.Bass(trn_type="TRN2")

    x_full = nc.dram_tensor("x_full", [S, H], f32, kind="ExternalInput")
    x_q = nc.dram_tensor("x_q", [NQ, H], f32, kind="ExternalInput")
    wq_d = nc.dram_tensor("wq", [H, H], bf16, kind="ExternalInput")
    wkv_d = nc.dram_tensor("wkv", [H, 256], bf16, kind="ExternalInput")
    wo_d = nc.dram_tensor("wo", [H, H], bf16, kind="ExternalInput")
    wfc_d = nc.dram_tensor("wfc", [H, INTER], bf16, kind="ExternalInput")
    wproj_d = nc.dram_tensor("wproj", [INTER, H], bf16, kind="ExternalInput")
    bq_d = nc.dram_tensor("bq", [H], f32, kind="ExternalInput")
    bkv_d = nc.dram_tensor("bkv", [256], f32, kind="ExternalInput")
    bo_d = nc.dram_tensor("bo", [H], f32, kind="ExternalInput")
    bfc_d = nc.dram_tensor("bfc", [INTER], f32, kind="ExternalInput")
    bproj_d = nc.dram_tensor("bproj", [H], f32, kind="ExternalInput")
    mask_d = nc.dram_tensor("mask", [128, 256], f32, kind="ExternalInput")
    out_d = nc.dram_tensor("out", [NQ, H], f32, kind="ExternalOutput")

    with TileContext(nc) as tc:
        with (
            tc.tile_pool(name="const", bufs=1) as constp,
            tc.tile_pool(name="big", bufs=1) as bigp,
            tc.tile_pool(name="b4", bufs=2) as b4p,
            tc.tile_pool(name="kvp", bufs=1) as kvp,
            tc.tile_pool(name="work", bufs=2) as workp,
            tc.tile_pool(name="psum", bufs=2, space="PSUM") as psump,
            tc.tile_pool(name="dram", bufs=1, space="DRAM") as dramp,
        ):
            # ---- constants ----
            id_f = constp.tile([128, 128], f32, name="id_f")
            make_identity(nc, id_f)
            id_b = constp.tile([128, 128], bf16, name="id_b")
            make_identity(nc, id_b)
            eps_t = constp.tile([128, 1], f32, name="eps_t")
            nc.vector.memset(eps_t, EPS)
            mask_sb = constp.tile([128, 256], f32, name="mask_sb")
            nc.sync.dma_start(mask_sb, mask_d[:, :])
            bq_sb = constp.tile([128, HT], f32, name="bq_sb")
            nc.sync.dma_start(bq_sb, bq_d.rearrange("(m p) -> p m", p=128))
            bkv_sb = constp.tile([128, 2], f32, name="bkv_sb")
            nc.sync.dma_start(bkv_sb, bkv_d.rearrange("(m p) -> p m", p=128))
            bo_sb = constp.tile([128, HT], f32, name="bo_sb")
            nc.sync.dma_start(bo_sb, bo_d.rearrange("(m p) -> p m", p=128))
            bfc_sb = constp.tile([128, IT], f32, name="bfc_sb")
            nc.sync.dma_start(bfc_sb, bfc_d.rearrange("(m p) -> p m", p=128))
            bproj_sb = constp.tile([128, HT], f32, name="bproj_sb")
            nc.sync.dma_start(bproj_sb, bproj_d.rearrange("(m p) -> p m", p=128))
            wkv_sb = constp.tile([128, HT, 256], bf16, name="wkv_sb")
            nc.sync.dma_start(wkv_sb, wkv_d.rearrange("(k p) n -> p k n", p=128))

            y_dram = dramp.tile([NQ, H], f32, name="y_dram")

            wq_r = wq_d.rearrange("(k p) n -> p k n", p=128)
            wo_r = wo_d.rearrange("(k p) n -> p k n", p=128)
            wfc_r = wfc_d.rearrange("(k p) n -> p k n", p=128)
            wproj_r = wproj_d.rearrange("(k p) n -> p k n", p=128)

            # ---- S1: LN1 over full sequence, transposed -> lnT ----
            lnT = bigp.tile([128, HT, S], bf16, tag="big8", name="lnT")
            for t in range(KT):
                x_t = workp.tile([128, H], f32, tag="xf", bufs=2, name="x_t")
                nc.sync.dma_start(x_t, x_full[t * 128:(t + 1) * 128, :])
                ln_t = workp.tile([128, H], bf16, tag="lnb", bufs=2, name="ln_t")
                _layernorm(nc, workp, x_t, ln_t, eps_t)
                for k in range(HT):
                    pt = psump.tile([128, 128], bf16, tag="tp", bufs=4, name="pt")
                    nc.tensor.transpose(pt, ln_t[:, k * 128:(k + 1) * 128], id_b)
                    nc.scalar.copy(lnT[:, k, t * 128:(t + 1) * 128], pt)

            # ---- S2: K/V heads (K pre-scaled on host) ----
            kT_sb = kvp.tile([128, S], bf16, name="kT_sb")
            vT_sb = workp.tile([128, S], bf16, tag="h", bufs=2, name="vT_sb")
            for m in range(2):
                for n4 in range(4):
                    pk = psump.tile([128, 512], f32, tag="sc", bufs=2, name="pk")
                    for k in range(HT):
                        nc.tensor.matmul(
                            pk, wkv_sb[:, k, m * 128:(m + 1) * 128],
                            lnT[:, k, n4 * 512:(n4 + 1) * 512],
                            start=(k == 0), stop=(k == HT - 1),
                        )
                    dst = kT_sb if m == 0 else vT_sb
                    nc.scalar.activation(
                        dst[:, n4 * 512:(n4 + 1) * 512], pk, AF.Identity,
                        bias=bkv_sb[:, m:m + 1],
                    )
            v_sb = kvp.tile([128, KT, 128], bf16, name="v_sb")
            for j in range(KT):
                pt = psump.tile([128, 128], bf16, tag="tp", bufs=4, name="pt")
                nc.tensor.transpose(pt, vT_sb[:, j * 128:(j + 1) * 128], id_b)
                nc.scalar.copy(v_sb[:, j, :], pt)

            # ---- S3: LN1 of own query tokens, transposed -> lnqT ----
            lnqT = b4p.tile([128, HT, NQ], bf16, tag="b4", name="lnqT")
            for t in range(QT):
                xq_t = workp.tile([128, H], f32, tag="xf", bufs=2, name="xq_t")
                nc.sync.dma_start(xq_t, x_q[t * 128:(t + 1) * 128, :])
                lnq_t = workp.tile([128, H], bf16, tag="lnb", bufs=2, name="lnq_t")
                _layernorm(nc, workp, xq_t, lnq_t, eps_t)
                for k in range(HT):
                    pt = psump.tile([128, 128], bf16, tag="tp", bufs=4, name="pt")
                    nc.tensor.transpose(pt, lnq_t[:, k * 128:(k + 1) * 128], id_b)
                    nc.scalar.copy(lnqT[:, k, t * 128:(t + 1) * 128], pt)

            # ---- S4: qT = wq^T @ lnqT (+bq) ----
            qT = b4p.tile([128, NH, NQ], bf16, tag="b4", name="qT")
            for m in range(HT):
                band = workp.tile([128, HT, 128], bf16, tag="band", bufs=3, name="band")
                nc.sync.dma_start(band, wq_r[:, :, m * 128:(m + 1) * 128])
                for half in range(2):
                    pq = psump.tile([128, 512], f32, tag="sc", bufs=2, name="pq")
                    for k in range(HT):
                        nc.tensor.matmul(
                            pq, band[:, k, :],
                            lnqT[:, k, half * 512:(half + 1) * 512],
                            start=(k == 0), stop=(k == HT - 1),
                        )
                    nc.scalar.activation(
                        qT[:, m, half * 512:(half + 1) * 512], pq, AF.Identity,
                        bias=bq_sb[:, m:m + 1],
                    )

            # ---- S5: attention (causal, static key-extent per q tile) ----
            attnT = b4p.tile([128, NH, NQ], bf16, tag="b4", name="attnT")
            for hd in range(NH):
                for i in range(QT):
                    nk = (2 * i + 2) * 128
                    nch = (nk + 511) // 512
                    probs = workp.tile([128, 2048], bf16, tag="h", bufs=2, name="probs")
                    den = workp.tile([128, 4], f32, tag="den", bufs=3, name="den")
                    for ch in range(nch):
                        w = min(512, nk - ch * 512)
                        ps = psump.tile([128, 512], f32, tag="sc", bufs=2, name="ps")
                        nc.tensor.matmul(
                            ps[:, :w], qT[:, hd, i * 128:(i + 1) * 128],
                            kT_sb[:, ch * 512:ch * 512 + w],
                            start=True, stop=True,
                        )
                        if ch == nch - 1:
                            nc.vector.tensor_add(
                                ps[:, w - 256:w], ps[:, w - 256:w], mask_sb
                            )
                        nc.scalar.activation(
                            probs[:, ch * 512:ch * 512 + w], ps[:, :w], AF.Exp,
                            accum_out=den[:, ch:ch + 1],
                        )
                    rec = workp.tile([128, 1], f32, tag="rec", bufs=3, name="rec")
                    if nch > 1:
                        nc.vector.reduce_sum(rec, den[:, :nch], axis=mybir.AxisListType.X)
                        nc.vector.reciprocal(rec, rec)
                    else:
                        nc.vector.reciprocal(rec, den[:, 0:1])
                    pa = psump.tile([128, 128], f32, tag="acc", bufs=2, name="pa")
                    nkt = 2 * i + 2
                    for kt in range(nkt):
                        ptp = psump.tile([128, 128], bf16, tag="tp", bufs=4, name="ptp")
                        nc.tensor.transpose(
                            ptp, probs[:, kt * 128:(kt + 1) * 128], id_b
                        )
                        pT = workp.tile([128, 128], bf16, tag="pT", bufs=4, name="pT")
                        if kt % 2:
                            nc.vector.tensor_copy(pT, ptp)
                        else:
                            nc.scalar.copy(pT, ptp)
                        nc.tensor.matmul(
                            pa, pT, v_sb[:, kt, :],
                            start=(kt == 0), stop=(kt == nkt - 1),
                        )
                    att = workp.tile([128, 128], bf16, tag="att", bufs=3, name="att")
                    nc.vector.tensor_scalar_mul(att, pa, rec)
                    pat = psump.tile([128, 128], bf16, tag="tp", bufs=4, name="pat")
                    nc.tensor.transpose(pat, att, id_b)
                    nc.scalar.copy(attnT[:, hd, i * 128:(i + 1) * 128], pat)

            # ---- S6: y = attn @ wo + bo + x_q  -> y_dram ----
            for m in range(HT):
                band = workp.tile([128, NH, 128], bf16, tag="band", bufs=3, name="band")
                nc.sync.dma_start(band, wo_r[:, :, m * 128:(m + 1) * 128])
                yTb = workp.tile([128, NQ], f32, tag="yT", bufs=2, name="yTb")
                for half in range(2):
                    py = psump.tile([128, 512], f32, tag="sc", bufs=2, name="py")
                    for k in range(NH):
                        nc.tensor.matmul(
                            py, band[:, k, :],
                            attnT[:, k, half * 512:(half + 1) * 512],
                            start=(k == 0), stop=(k == NH - 1),
                        )
                    nc.scalar.activation(
                        yTb[:, half * 512:(half + 1) * 512], py, AF.Identity,
                        bias=bo_sb[:, m:m + 1],
                    )
                for t in range(QT):
                    ptp = psump.tile([128, 128], f32, tag="tp", bufs=4, name="ptp")
                    nc.tensor.transpose(ptp, yTb[:, t * 128:(t + 1) * 128], id_f)
                    rb = workp.tile([128, 128], f32, tag="xqb", bufs=4, name="rb")
                    nc.sync.dma_start(
                        rb, x_q[t * 128:(t + 1) * 128, m * 128:(m + 1) * 128]
                    )
                    yb = workp.tile([128, 128], f32, tag="yb", bufs=4, name="yb")
                    nc.vector.tensor_add(yb, ptp, rb)
                    nc.sync.dma_start(
                        y_dram[t * 128:(t + 1) * 128, m * 128:(m + 1) * 128], yb
                    )

            # ---- S7: LN2, transposed -> ln2T ----
            ln2T = b4p.tile([128, HT, NQ], bf16, tag="b4", name="ln2T")
            for t in range(QT):
                y_t = workp.tile([128, H], f32, tag="xf", bufs=2, name="y_t")
                nc.sync.dma_start(y_t, y_dram[t * 128:(t + 1) * 128, :])
                ln2_t = workp.tile([128, H], bf16, tag="lnb", bufs=2, name="ln2_t")
                _layernorm(nc, workp, y_t, ln2_t, eps_t)
                for k in range(HT):
                    pt = psump.tile([128, 128], bf16, tag="tp", bufs=4, name="pt")
                    nc.tensor.transpose(pt, ln2_t[:, k * 128:(k + 1) * 128], id_b)
                    nc.scalar.copy(ln2T[:, k, t * 128:(t + 1) * 128], pt)

            # ---- S8/S9: MLP in two token halves (gT fits one 8MB slot) ----
            for hq in range(2):
                tok0 = hq * 512
                gT = bigp.tile([128, IT, 512], bf16, tag="big8", name="gT")
                for mi in range(IT):
                    band = workp.tile([128, HT, 128], bf16, tag="band", bufs=3, name="band")
                    nc.sync.dma_start(band, wfc_r[:, :, mi * 128:(mi + 1) * 128])
                    pf = psump.tile([128, 512], f32, tag="sc", bufs=2, name="pf")
                    for k in range(HT):
                        nc.tensor.matmul(
                            pf, band[:, k, :], ln2T[:, k, tok0:tok0 + 512],
                            start=(k == 0), stop=(k == HT - 1),
                        )
                    nc.scalar.activation(
                        gT[:, mi, :], pf, AF.Gelu_apprx_tanh,
                        bias=bfc_sb[:, mi:mi + 1],
                    )
                for m in range(HT):
                    po = psump.tile([128, 512], f32, tag="sc", bufs=2, name="po")
                    for kg in range(4):
                        band = workp.tile(
                            [128, HT, 128], bf16, tag="band", bufs=3, name="band"
                        )
                        nc.sync.dma_start(
                            band, wproj_r[:, kg * 16:(kg + 1) * 16, m * 128:(m + 1) * 128]
                        )
                        for kk in range(HT):
                            k = kg * 16 + kk
                            nc.tensor.matmul(
                                po, band[:, kk, :], gT[:, k, :],
                                start=(k == 0), stop=(k == IT - 1),
                            )
                    oT = workp.tile([128, 512], f32, tag="yT", bufs=2, name="oT")
                    nc.scalar.activation(
                        oT, po, AF.Identity, bias=bproj_sb[:, m:m + 1]
                    )
                    for tt in range(4):
                        t = hq * 4 + tt
                        ptp = psump.tile([128, 128], f32, tag="tp", bufs=4, name="ptp")
                        nc.tensor.transpose(ptp, oT[:, tt * 128:(tt + 1) * 128], id_f)
                        rb = workp.tile([128, 128], f32, tag="xqb", bufs=4, name="rb")
                        nc.sync.dma_start(
                            rb, y_dram[t * 128:(t + 1) * 128, m * 128:(m + 1) * 128]
                        )
                        yb = workp.tile([128, 128], f32, tag="yb", bufs=4, name="yb")
                        nc.vector.tensor_add(yb, ptp, rb)
                        nc.sync.dma_start(
                            out_d[t * 128:(t + 1) * 128, m * 128:(m + 1) * 128], yb
                        )
    _split_excess_waits(nc)
    return nc


_PROG = None


def _get_prog():
    global _PROG
    if _PROG is None:
        _PROG = _build_program()
    return _PROG


def kernel(hidden_states, ln1_g, ln1_b, ln2_g, ln2_b, wq, bq, wkv, bkv,
           wo, bo, wfc, bfc, wproj, bproj):
    hs = np.asarray(hidden_states, np.float32)
    ln1_g = np.asarray(ln1_g, np.float32)
    ln1_b = np.asarray(ln1_b, np.float32)
    ln2_g = np.asarray(ln2_g, np.float32)
    ln2_b = np.asarray(ln2_b, np.float32)
    wq = np.asarray(wq, np.float32)
    wkv = np.asarray(wkv, np.float32)
    wo = np.asarray(wo, np.float32)
    wfc = np.asarray(wfc, np.float32)
    wproj = np.asarray(wproj, np.float32)

    # Fold LN gain/bias into the following matmuls; fold qk scale into K.
    wq_e = ln1_g[:, None] * wq
    bq_e = np.asarray(bq, np.float32) + ln1_b @ wq
    wkv_e = (ln1_g[:, None] * wkv).copy()
    bkv_e = (np.asarray(bkv, np.float32) + ln1_b @ wkv).copy()
    scale = 1.0 / np.sqrt(D)
    wkv_e[:, :D] *= scale
    bkv_e[:D] *= scale
    wfc_e = ln2_g[:, None] * wfc
    bfc_e = np.asarray(bfc, np.float32) + ln2_b @ wfc

    def to_bf(a):
        return np.ascontiguousarray(a.astype(ml_dtypes.bfloat16))

    wq_b, wkv_b, wo_b = to_bf(wq_e), to_bf(wkv_e), to_bf(wo)
    wfc_b, wproj_b = to_bf(wfc_e), to_bf(wproj)
    bo_f = np.ascontiguousarray(np.asarray(bo, np.float32))
    bproj_f = np.ascontiguousarray(np.asarray(bproj, np.float32))
    bq_e = np.ascontiguousarray(bq_e)
    bfc_e = np.ascontiguousarray(bfc_e)

    tril = np.where(np.tril(np.ones((128, 128), bool)), 0.0, NEG).astype(np.float32)
    mask_h = [
        np.ascontiguousarray(
            np.concatenate([tril, np.full((128, 128), NEG, np.float32)], axis=1)),
        np.ascontiguousarray(
            np.concatenate([np.zeros((128, 128), np.float32), tril], axis=1)),
    ]

    in_maps = []
    for c in range(8):
        b, h = divmod(c, 2)
        xb = np.ascontiguousarray(hs[b])
        xq = np.ascontiguousarray(xb.reshape(8, 2, 128, H)[:, h].reshape(NQ, H))
        in_maps.append(dict(
            x_full=xb, x_q=xq, wq=wq_b, wkv=wkv_b, wo=wo_b, wfc=wfc_b,
            wproj=wproj_b, bq=bq_e, bkv=bkv_e, bo=bo_f, bfc=bfc_e,
            bproj=bproj_f, mask=mask_h[h],
        ))

    global last_in_maps
    last_in_maps = in_maps
    res = run_bass_kernel_spmd(_get_prog(), in_maps, core_ids=list(range(8)))
    kernel.last_result = res

    out = np.empty((B, S, H), np.float32)
    for c in range(8):
        b, h = divmod(c, 2)
        out[b].reshape(8, 2, 128, H)[:, h] = (
            np.asarray(res.results[c]["out"]).reshape(8, 128, H)
        )
    return out



# revision 2
# speedup vs baseline: 1.0089x; 1.0089x over previous
"""GPTBigCode transformer block (MQA) on 8 trn2 NeuronCores — v2.

Sharding: data-parallel over batch (4) x parity-interleaved q-block split
(2) per batch element. Core c handles batch c//2 and q-blocks {2j + c%2}.
No collectives; K/V (single MQA head) recomputed per core.

v2 keeps ALL activations feature-on-partition ("T layout") end-to-end —
zero PE transposes. LayerNorm statistics are computed with ones-vector
matmul chains (partition-axis reduction on the tensor engine), per-token
scalars are broadcast back across partitions with K=1 matmuls. Attention
computes transposed scores (keys-on-partition) so softmax-denominators
come from ones-matmuls and probs feed attn@V directly. The softmax
normalization is applied as a per-column multiply on the attention
output. Causal masking of the parity-dependent diagonal zone uses two
per-core mask inputs so the compiled program is identical on all cores.

Weights are host-packed so every weight DMA is contiguous per partition
line; activations never round-trip through DRAM. Matmul inputs bf16;
accumulation, softmax and residual math f32 (residual stream bf16).
"""

import numpy as np
import ml_dtypes

# ---------------------------------------------------------------------------
# Workaround: this container's walrus build rejects >1 sync-wait on
# CTRL-class (Drain) instructions. Split the Tile tail-drain's waits into
# individual wait-carrying NOPs on the SP engine.
import bass_rust
from concourse.tile import TileContext
from concourse.vector_clock import ScopedClock


def _patched_drain_and_barrier(self, tick_clock, wait_clock):
    nc = self.nc
    drain_inst = nc.sync.drain()
    wait_clock.add_sem_waits(
        drain_inst.ins, ScopedClock({None: tick_clock.global_clock})
    )
    si = drain_inst.ins.sync_info
    waits = list(si.on_wait) if si and si.on_wait else []
    if len(waits) > 1:
        drain_inst.ins.sync_info = bass_rust.SyncInfo(
            on_wait=waits[:1],
            on_update=list(si.on_update) if si.on_update else [],
        )
        for w in waits[1:]:
            n = nc.sync.nop(nofuse=True, hint="split_drain_wait")
            n.ins.sync_info = bass_rust.SyncInfo(on_wait=[w], on_update=[])
    nc.all_engine_barrier()
    assert self.sems is not None
    popped = nc._tile_sem_poison_stack.pop()
    assert popped is self._sem_poison
    nc.clear_and_free_semaphores(list(self.sems.allocated().values()))
    nc.all_engine_barrier()


TileContext._drain_and_barrier = _patched_drain_and_barrier


def _split_excess_waits(nc, max_waits=1):
    """Rewrite every instruction carrying more than `max_waits` sem-waits:
    excess waits move onto same-engine NOPs inserted just before it."""
    all_bbs = [bb for fn in nc.m.functions for bb in fn.blocks]
    for bb in all_bbs:
        insts = list(bb.instructions)
        new_list = []
        changed = False
        for inst in insts:
            si = inst.sync_info
            waits = list(si.on_wait) if si and si.on_wait else []
            if len(waits) > max_waits:
                changed = True
                inst.sync_info = bass_rust.SyncInfo(
                    on_wait=waits[:max_waits],
                    on_update=list(si.on_update) if si.on_update else [],
                )
                for w in waits[max_waits:]:
                    nop_bi = nc.engines[inst.engine].nop(
                        nofuse=True, hint="wsplit"
                    )
                    nop = nop_bi.ins
                    cur = nc.cur_bb.bb
                    cl = list(cur.instructions)
                    assert cl and cl[-1].name == nop.name, "nop not appended last"
                    cur.instructions = cl[:-1]
                    nop.sync_info = bass_rust.SyncInfo(on_wait=[w], on_update=[])
                    new_list.append(nop)
            new_list.append(inst)
        if changed:
            bb.instructions = new_list
# ---------------------------------------------------------------------------

import concourse.bass as bass
import concourse.mybir as mybir
from concourse.bass_utils import run_bass_kernel_spmd
from concourse.masks import make_identity

f32 = mybir.dt.float32
bf16 = mybir.dt.bfloat16
AF = mybir.ActivationFunctionType
ALU = mybir.AluOpType

H = 2048
NH = 16
D = 128
INTER = 8192
S = 2048
B = 4
NQ = 1024          # query tokens per core
HT = H // 128      # 16
IT = INTER // 128  # 64
NCH = S // 512     # 4 full-seq chunks
EPS = 1e-5
NEG = -30000.0
INV_H = 1.0 / H


def _ln_rows(nc, rowp, psB, sum_ps, sumsq_ps, eps_t):
    """[1,512] psum sums -> (m_sb f32, rstd_sb f32) row tiles."""
    m_sb = rowp.tile([1, 512], f32, tag="rows", bufs=3, name="m_sb")
    nc.scalar.mul(m_sb, sum_ps, INV_H)
    v_sb = rowp.tile([1, 512], f32, tag="rows", bufs=3, name="v_sb")
    nc.scalar.mul(v_sb, sumsq_ps, INV_H)
    m2 = rowp.tile([1, 512], f32, tag="rows", bufs=3, name="m2")
    nc.vector.tensor_mul(m2, m_sb, m_sb)
    nc.vector.tensor_sub(v_sb, v_sb, m2)
    # rstd = exp(-0.5*ln(var+eps)) — keeps the whole row path on ScalarE
    # (DVE reciprocal on a 1-partition row is ~3.3us serial).
    nc.scalar.activation(v_sb, v_sb, AF.Ln, bias=eps_t)
    rstd = rowp.tile([1, 512], f32, tag="rows", bufs=3, name="rstd")
    nc.scalar.activation(rstd, v_sb, AF.Exp, scale=-0.5)
    return m_sb, rstd


def _ln_chunk_stats(nc, workp, rowp, psB, xc, ones_col, eps_t):
    """LN stats for one [128,16,512] bf16 chunk (raw x, T layout)."""
    sum_ps = psB.tile([1, 512], f32, tag="B", bufs=2, name="sum_ps")
    sumsq_ps = psB.tile([1, 512], f32, tag="B", bufs=2, name="sumsq_ps")
    for kt in range(HT):
        nc.tensor.matmul(sum_ps, ones_col, xc[:, kt, :],
                         start=(kt == 0), stop=(kt == HT - 1))
    for kt in range(HT):
        sq = workp.tile([128, 512], bf16, tag="t2k", bufs=3, name="sq")
        nc.scalar.square(sq, xc[:, kt, :])
        nc.tensor.matmul(sumsq_ps, ones_col, sq,
                         start=(kt == 0), stop=(kt == HT - 1))
    return _ln_rows(nc, rowp, psB, sum_ps, sumsq_ps, eps_t)


def _bcast_row_bf16(nc, workp, psC, ones_rowf, row_sb, name):
    """[1,512] f32 row -> [128,512] bf16 sbuf broadcast tile."""
    bc_ps = psC.tile([128, 512], f32, tag="C", bufs=2, name=f"{name}_ps")
    nc.tensor.matmul(bc_ps, ones_rowf, row_sb, start=True, stop=True)
    bc_sb = workp.tile([128, 512], bf16, tag="bc1k", bufs=2, name=f"{name}_sb")
    nc.scalar.copy(bc_sb, bc_ps)
    return bc_sb


def _scale_inplace(nc, xc, bcr):
    """xc *= rstd (per column). The mean is folded into the following
    matmul chains as a K=1 rank-1 correction with -colsum(W)."""
    for kt in range(HT):
        nc.vector.tensor_mul(xc[:, kt, :], xc[:, kt, :], bcr)


def _build_program():
    nc = bass.Bass(trn_type="TRN2")

    xt_d = nc.dram_tensor("xt", [NCH, 128, HT, 512], bf16, kind="ExternalInput")
    xq_d = nc.dram_tensor("xq", [2, 128, HT, 512], bf16, kind="ExternalInput")
    xtq_d = nc.dram_tensor("xtq", [HT, 2, 128, 512], bf16, kind="ExternalInput")
    wq_d = nc.dram_tensor("wq", [NH, 128, HT, 128], bf16, kind="ExternalInput")
    wk_d = nc.dram_tensor("wk", [128, HT, 128], bf16, kind="ExternalInput")
    wv_d = nc.dram_tensor("wv", [128, HT, 128], bf16, kind="ExternalInput")
    wo_d = nc.dram_tensor("wo", [HT, 128, HT, 128], bf16, kind="ExternalInput")
    wfc_d = nc.dram_tensor("wfc", [IT, 128, HT, 128], bf16, kind="ExternalInput")
    wproj_d = nc.dram_tensor("wproj", [HT, 128, IT, 128], bf16, kind="ExternalInput")
    bq_d = nc.dram_tensor("bq", [128, NH], f32, kind="ExternalInput")
    bk_d = nc.dram_tensor("bk", [128, 1], f32, kind="ExternalInput")
    bv_d = nc.dram_tensor("bv", [1, 128], bf16, kind="ExternalInput")
    bo_d = nc.dram_tensor("bo", [128, HT], f32, kind="ExternalInput")
    bfc_d = nc.dram_tensor("bfc", [128, IT], f32, kind="ExternalInput")
    bproj_d = nc.dram_tensor("bproj", [128, HT], f32, kind="ExternalInput")
    maskA_d = nc.dram_tensor("maskA", [128, 512], bf16, kind="ExternalInput")
    maskB_d = nc.dram_tensor("maskB", [128, 512], bf16, kind="ExternalInput")
    csq_d = nc.dram_tensor("csq", [1, NH, 128], bf16, kind="ExternalInput")
    csk_d = nc.dram_tensor("csk", [1, 128], bf16, kind="ExternalInput")
    csv_d = nc.dram_tensor("csv", [1, 128], bf16, kind="ExternalInput")
    out_d = nc.dram_tensor("out", [H, NQ], f32, kind="ExternalOutput")

    with TileContext(nc) as tc:
        with (
            tc.tile_pool(name="const", bufs=1) as constp,
            tc.tile_pool(name="big", bufs=1) as bigp,
            tc.tile_pool(name="s32", bufs=2) as s32p,
            tc.tile_pool(name="str16", bufs=2) as strp,
            tc.tile_pool(name="band", bufs=3) as bandp,
            tc.tile_pool(name="work", bufs=2) as workp,
            tc.tile_pool(name="rows", bufs=4) as rowp,
            tc.tile_pool(name="psA", bufs=2, space="PSUM") as psA,
            tc.tile_pool(name="psB", bufs=2, space="PSUM") as psB,
            tc.tile_pool(name="psC", bufs=2, space="PSUM") as psC,
        ):
            # First input chunk DMA goes out before the constant loads so
            # compute can start as early as possible.
            xc0 = strp.tile([128, HT, 512], bf16, tag="str16", name="xc")
            nc.sync.dma_start(xc0, xt_d[0])

            # ---- constants ----
            ones_col = constp.tile([128, 1], bf16, name="ones_col")
            nc.vector.memset(ones_col, 1.0)
            ones_rowf = constp.tile([1, 128], f32, name="ones_rowf")
            nc.vector.memset(ones_rowf, 1.0)
            ones_rowb = constp.tile([1, 128], bf16, name="ones_rowb")
            nc.vector.memset(ones_rowb, 1.0)
            eps_t = constp.tile([1, 1], f32, name="eps_t")
            nc.vector.memset(eps_t, EPS)
            bq_sb = constp.tile([128, NH], f32, name="bq_sb")
            nc.sync.dma_start(bq_sb, bq_d[:, :])
            bk_sb = constp.tile([128, 1], f32, name="bk_sb")
            nc.sync.dma_start(bk_sb, bk_d[:, :])
            bv_sb = constp.tile([1, 128], bf16, name="bv_sb")
            nc.sync.dma_start(bv_sb, bv_d[:, :])
            bo_sb = constp.tile([128, HT], f32, name="bo_sb")
            nc.sync.dma_start(bo_sb, bo_d[:, :])
            bfc_sb = constp.tile([128, IT], f32, name="bfc_sb")
            nc.sync.dma_start(bfc_sb, bfc_d[:, :])
            bproj_sb = constp.tile([128, HT], f32, name="bproj_sb")
            nc.sync.dma_start(bproj_sb, bproj_d[:, :])
            maskA = constp.tile([128, 512], bf16, name="maskA")
            nc.sync.dma_start(maskA, maskA_d[:, :])
            maskB = constp.tile([128, 512], bf16, name="maskB")
            nc.sync.dma_start(maskB, maskB_d[:, :])
            csq_sb = constp.tile([1, NH, 128], bf16, name="csq_sb")
            nc.sync.dma_start(csq_sb, csq_d[:, :, :])
            csk_sb = constp.tile([1, 128], bf16, name="csk_sb")
            nc.sync.dma_start(csk_sb, csk_d[:, :])
            csv_sb = constp.tile([1, 128], bf16, name="csv_sb")
            nc.sync.dma_start(csv_sb, csv_d[:, :])
            id_bf = constp.tile([128, 128], bf16, name="id_bf")
            make_identity(nc, id_bf)
            wk_sb = bandp.tile([128, HT, 128], bf16, tag="band4", bufs=3,
                               name="wk_sb")
            nc.sync.dma_start(wk_sb, wk_d[:, :, :])
            wv_sb = bandp.tile([128, HT, 128], bf16, tag="band4", bufs=3,
                               name="wv_sb")
            nc.sync.dma_start(wv_sb, wv_d[:, :, :])
            kT_sb = constp.tile([128, S], bf16, name="kT_sb")
            vtm = constp.tile([128, HT, 128], bf16, name="vtm")

            # ---- phase A: full-seq LN1 + K/V, streamed in 512-token chunks
            for ci in range(NCH):
                if ci == 0:
                    xc = xc0
                else:
                    xc = strp.tile([128, HT, 512], bf16, tag="str16", name="xc")
                    nc.sync.dma_start(xc, xt_d[ci])
                m_sb, rstd = _ln_chunk_stats(nc, workp, rowp, psB, xc,
                                             ones_col, eps_t)
                bcr = _bcast_row_bf16(nc, workp, psC, ones_rowf, rstd, "bcr")
                mr = rowp.tile([1, 512], bf16, tag="mrow", bufs=2, name="mr")
                nc.vector.tensor_mul(mr, m_sb, rstd)
                _scale_inplace(nc, xc, bcr)
                # K^T chunk: [dk=128, 512 tokens]
                kps = psA.tile([128, 2, 512], f32, tag="A", bufs=2, name="kps")
                for kt in range(HT):
                    nc.tensor.matmul(kps[:, 0, :], wk_sb[:, kt, :], xc[:, kt, :],
                                     start=(kt == 0), stop=False)
                nc.tensor.matmul(kps[:, 0, :], csk_sb, mr,
                                 start=False, stop=True)
                nc.scalar.activation(kT_sb[:, ci * 512:(ci + 1) * 512],
                                     kps[:, 0, :], AF.Identity, bias=bk_sb)
                # V token-major: 4 token-blocks
                for tb in range(4):
                    vps = psA.tile([128, 2, 512], f32, tag="A", bufs=2, name="vps")
                    for kt in range(HT):
                        nc.tensor.matmul(
                            vps[:, 0, 0:128],
                            xc[:, kt, tb * 128:(tb + 1) * 128],
                            wv_sb[:, kt, :], start=(kt == 0), stop=False,
                        )
                    nc.tensor.matmul(vps[:, 0, 0:128], ones_rowb, bv_sb,
                                     start=False, stop=False)
                    nc.tensor.matmul(vps[:, 0, 0:128],
                                     mr[:, tb * 128:(tb + 1) * 128], csv_sb,
                                     start=False, stop=True)
                    nc.vector.tensor_copy(vtm[:, ci * 4 + tb, :], vps[:, 0, 0:128])

            # ---- phase B: own-q LN1 + Q projection (2 chunks of 512) ----
            # qT layout [dq, hg, j, hh, q]: scores rhs [:, hg, j] is a fully
            # contiguous 512-column block (4 heads x 128 q).
            qT = s32p.tile([128, 4, 8, 4, 128], bf16, tag="s32", name="qT")
            for ch in range(2):
                xqc = strp.tile([128, HT, 512], bf16, tag="str16", name="xqc")
                nc.sync.dma_start(xqc, xq_d[ch])
                m_sb, rstd = _ln_chunk_stats(nc, workp, rowp, psB, xqc,
                                             ones_col, eps_t)
                bcr = _bcast_row_bf16(nc, workp, psC, ones_rowf, rstd, "qbcr")
                mr = rowp.tile([1, 512], bf16, tag="mrow", bufs=2, name="qmr")
                nc.vector.tensor_mul(mr, m_sb, rstd)
                _scale_inplace(nc, xqc, bcr)
                for m in range(NH):
                    hg, hh = divmod(m, 4)
                    band = bandp.tile([128, HT, 128], bf16, tag="band4",
                                      bufs=3, name="band")
                    nc.sync.dma_start(band, wq_d[m])
                    qps = psA.tile([128, 2, 512], f32, tag="A", bufs=2, name="qps")
                    for kt in range(HT):
                        nc.tensor.matmul(qps[:, 0, :], band[:, kt, :],
                                         xqc[:, kt, :],
                                         start=(kt == 0), stop=False)
                    nc.tensor.matmul(qps[:, 0, :], csq_sb[:, m, :], mr,
                                     start=False, stop=True)
                    nc.scalar.activation(qT[:, hg, 4 * ch:4 * ch + 4, hh, :],
                                         qps[:, 0, :], AF.Identity,
                                         bias=bq_sb[:, m:m + 1])

            # ---- phase C: attention (scoresT, padded extent E=2j+2) ----
            # Two q-blocks' ladders are interleaved per head-group so the PE
            # fills the ACT-exp latency of one block with the other block's
            # matmuls.
            attnT = s32p.tile([128, NH, NQ], bf16, tag="s32", name="attnT")

            def _attn_step(hg, j, idx, kts, den_ps, av_ps):
                E = len(kts)
                kt = kts[idx]
                masked = kt >= E - 2
                sc = psA.tile([128, 2, 512], f32, tag="A", bufs=2, name="sc")
                nc.tensor.matmul(
                    sc[:, 0, :], kT_sb[:, kt * 128:(kt + 1) * 128],
                    qT[:, hg, j], start=True, stop=not masked,
                )
                if masked:
                    # mask add on the PE: sc += I^T @ mask
                    nc.tensor.matmul(sc[:, 0, :], id_bf,
                                     maskA if kt == E - 2 else maskB,
                                     start=False, stop=True)
                ex = workp.tile([128, 512], bf16, tag="t2k", bufs=3, name="ex")
                nc.scalar.activation(ex, sc[:, 0, :], AF.Exp)
                nc.tensor.matmul(den_ps, ones_col, ex,
                                 start=(idx == 0), stop=(idx == E - 1))
                nc.tensor.matmul(av_ps, vtm[:, kt, :], ex,
                                 start=(idx == 0), stop=(idx == E - 1))

            def _attn_tail(hg, j, den_ps, av_ps):
                lnd = rowp.tile([1, 512], f32, tag="rows", bufs=3, name="lnd")
                nc.scalar.activation(lnd, den_ps, AF.Ln)
                rec = rowp.tile([1, 512], f32, tag="rows", bufs=3, name="rec")
                nc.scalar.activation(rec, lnd, AF.Exp, scale=-1.0)
                bcr_ps = psB.tile([128, 512], f32, tag="B", bufs=2,
                                  name="bcr_ps")
                nc.tensor.matmul(bcr_ps, ones_rowf, rec, start=True, stop=True)
                bcr_sb = workp.tile([128, 512], bf16, tag="bc1k", bufs=2,
                                    name="bcr_sb")
                nc.scalar.copy(bcr_sb, bcr_ps)
                nc.vector.tensor_mul(
                    attnT[:, hg * 4:(hg + 1) * 4, j * 128:(j + 1) * 128],
                    av_ps, bcr_sb,
                )

            for hg in range(4):
                for jp in range(4):
                    j0, j1 = 2 * jp, 2 * jp + 1
                    E0, E1 = 2 * j0 + 2, 2 * j1 + 2
                    kts0 = [E0 - 2, E0 - 1] + list(range(E0 - 2))
                    kts1 = [E1 - 2, E1 - 1] + list(range(E1 - 2))
                    den0 = psB.tile([1, 512], f32, tag="B", bufs=2, name="den0")
                    av0 = psC.tile([128, 512], f32, tag="C", bufs=2, name="av0")
                    den1 = psB.tile([1, 512], f32, tag="B", bufs=2, name="den1")
                    av1 = psC.tile([128, 512], f32, tag="C", bufs=2, name="av1")
                    for i in range(E1):
                        if i < E0:
                            _attn_step(hg, j0, i, kts0, den0, av0)
                        elif i == E0:
                            _attn_tail(hg, j0, den0, av0)
                        _attn_step(hg, j1, i, kts1, den1, av1)
                    _attn_tail(hg, j1, den1, av1)

            # ---- phase D: out-proj + residual -> y ----
            # ch-outer: y[:, :, ch0] completes early so LN2+fc of chunk 0
            # overlap the second wo half.
            y = s32p.tile([128, HT, NQ], bf16, tag="s32", name="y")
            for ch in range(2):
                for ob in range(HT):
                    band = bandp.tile([128, HT, 128], bf16, tag="band4", bufs=3,
                                      name="band")
                    nc.sync.dma_start(band, wo_d[ob])
                    wps = psA.tile([128, 2, 512], f32, tag="A", bufs=2,
                                   name="wps")
                    for ht in range(HT):
                        nc.tensor.matmul(wps[:, 0, :], band[:, ht, :],
                                         attnT[:, ht, ch * 512:(ch + 1) * 512],
                                         start=(ht == 0), stop=(ht == HT - 1))
                    xqt = workp.tile([128, 512], bf16, tag="xq1k", bufs=2,
                                     name="xqt")
                    nc.sync.dma_start(xqt, xtq_d[ob, ch])
                    nc.vector.scalar_tensor_tensor(
                        out=y[:, ob, ch * 512:(ch + 1) * 512],
                        in0=wps[:, 0, :], scalar=bo_sb[:, ob:ob + 1],
                        in1=xqt, op0=ALU.add, op1=ALU.add,
                    )

            # ---- phase E: LN2 + MLP + residual -> out (per 512-token chunk)
            for ch in range(2):
                cols = slice(ch * 512, (ch + 1) * 512)
                sum_ps = psB.tile([1, 512], f32, tag="B", bufs=2, name="l2sum")
                sumsq_ps = psB.tile([1, 512], f32, tag="B", bufs=2, name="l2sq")
                for kt in range(HT):
                    nc.tensor.matmul(sum_ps, ones_col, y[:, kt, cols],
                                     start=(kt == 0), stop=(kt == HT - 1))
                for kt in range(HT):
                    sq = workp.tile([128, 512], bf16, tag="t2k", bufs=3,
                                    name="sq2")
                    nc.scalar.square(sq, y[:, kt, cols])
                    nc.tensor.matmul(sumsq_ps, ones_col, sq,
                                     start=(kt == 0), stop=(kt == HT - 1))
                m_sb, rstd = _ln_rows(nc, rowp, psB, sum_ps, sumsq_ps, eps_t)
                bcm = _bcast_row_bf16(nc, workp, psC, ones_rowf, m_sb, "l2bcm")
                bcr = _bcast_row_bf16(nc, workp, psC, ones_rowf, rstd, "l2bcr")
                ln2s = s32p.tile([128, HT, 512], bf16, tag="s32", name="ln2s")
                for kt in range(HT):
                    nc.vector.tensor_sub(ln2s[:, kt, :], y[:, kt, cols], bcm)
                    nc.vector.tensor_mul(ln2s[:, kt, :], ln2s[:, kt, :], bcr)

                gT = bigp.tile([128, IT, 512], bf16, tag="big", name="gT")
                for mb in range(IT):
                    band = bandp.tile([128, HT, 128], bf16, tag="band4",
                                      bufs=3, name="band")
                    nc.sync.dma_start(band, wfc_d[mb])
                    fps = psA.tile([128, 2, 512], f32, tag="A", bufs=2,
                                   name="fps")
                    for kt in range(HT):
                        nc.tensor.matmul(fps[:, 0, :], band[:, kt, :],
                                         ln2s[:, kt, :],
                                         start=(kt == 0), stop=(kt == HT - 1))
                    nc.scalar.activation(gT[:, mb, :], fps[:, 0, :],
                                         AF.Gelu_apprx_tanh,
                                         bias=bfc_sb[:, mb:mb + 1])

                for ob in range(HT):
                    pband = strp.tile([128, IT, 128], bf16, tag="str16",
                                      name="pband")
                    nc.sync.dma_start(pband, wproj_d[ob])
                    pps = psA.tile([128, 2, 512], f32, tag="A", bufs=2,
                                   name="pps")
                    for mt in range(IT):
                        nc.tensor.matmul(pps[:, 0, :], pband[:, mt, :],
                                         gT[:, mt, :],
                                         start=(mt == 0), stop=(mt == IT - 1))
                    osb = workp.tile([128, 512], f32, tag="f2k", bufs=2,
                                     name="osb")
                    nc.vector.scalar_tensor_tensor(
                        out=osb, in0=pps[:, 0, :],
                        scalar=bproj_sb[:, ob:ob + 1],
                        in1=y[:, ob, cols], op0=ALU.add, op1=ALU.add,
                    )
                    nc.sync.dma_start(
                        out_d[ob * 128:(ob + 1) * 128, cols], osb
                    )
    _split_excess_waits(nc)
    return nc


_PROG = None


def _get_prog():
    global _PROG
    if _PROG is None:
        _PROG = _build_program()
    return _PROG


def _to_bf(a):
    return np.ascontiguousarray(a.astype(ml_dtypes.bfloat16))


def kernel(hidden_states, ln1_g, ln1_b, ln2_g, ln2_b, wq, bq, wkv, bkv,
           wo, bo, wfc, bfc, wproj, bproj):
    hs = np.asarray(hidden_states, np.float32)
    ln1_g = np.asarray(ln1_g, np.float32)
    ln1_b = np.asarray(ln1_b, np.float32)
    ln2_g = np.asarray(ln2_g, np.float32)
    ln2_b = np.asarray(ln2_b, np.float32)
    wq = np.asarray(wq, np.float32)
    wkv = np.asarray(wkv, np.float32)
    wo = np.asarray(wo, np.float32)
    wfc = np.asarray(wfc, np.float32)
    wproj = np.asarray(wproj, np.float32)

    # Fold LN gains into the following matmuls; fold qk scale into K.
    wq_e = ln1_g[:, None] * wq
    bq_e = np.asarray(bq, np.float32) + ln1_b @ wq
    wkv_e = ln1_g[:, None] * wkv
    bkv_e = np.asarray(bkv, np.float32) + ln1_b @ wkv
    scale = 1.0 / np.sqrt(D)
    wk_e = wkv_e[:, :D] * scale
    bk_e = bkv_e[:D] * scale
    wv_e = wkv_e[:, D:]
    bv_e = bkv_e[D:]
    wfc_e = ln2_g[:, None] * wfc
    bfc_e = np.asarray(bfc, np.float32) + ln2_b @ wfc

    # Host-packed weight layouts: [out-block, partition, k-tile, n] so each
    # band DMA is contiguous per partition line.
    wq_l = _to_bf(wq_e.reshape(HT, 128, NH, 128).transpose(2, 1, 0, 3))
    wk_l = _to_bf(wk_e.reshape(HT, 128, 128).transpose(1, 0, 2))
    wv_l = _to_bf(wv_e.reshape(HT, 128, 128).transpose(1, 0, 2))
    wo_l = _to_bf(wo.reshape(HT, 128, HT, 128).transpose(2, 1, 0, 3))
    wfc_l = _to_bf(wfc_e.reshape(HT, 128, IT, 128).transpose(2, 1, 0, 3))
    wproj_l = _to_bf(wproj.reshape(IT, 128, HT, 128).transpose(2, 1, 0, 3))

    # Negated column sums for the K=1 LN-mean-fold correction matmuls.
    csq_r = _to_bf(-wq_e.sum(axis=0).reshape(1, NH, 128))
    csk_r = _to_bf(-wk_e.sum(axis=0)[None, :])
    csv_r = _to_bf(-wv_e.sum(axis=0)[None, :])

    bq_r = np.ascontiguousarray(bq_e.reshape(NH, 128).T)
    bk_r = np.ascontiguousarray(bk_e[:, None])
    bv_r = _to_bf(bv_e[None, :])
    bo_r = np.ascontiguousarray(np.asarray(bo, np.float32).reshape(HT, 128).T)
    bfc_r = np.ascontiguousarray(bfc_e.reshape(IT, 128).T)
    bproj_r = np.ascontiguousarray(
        np.asarray(bproj, np.float32).reshape(HT, 128).T)

    # Causal masks for the two parity-dependent diagonal k-tiles.
    tri = np.where(np.arange(128)[None, :] >= np.arange(128)[:, None],
                   0.0, NEG).astype(np.float32)          # [k,q]
    tri4 = np.tile(tri, (1, 4))                          # [128, 512] (4 heads)
    zeros4 = np.zeros((128, 512), np.float32)
    neg4 = np.full((128, 512), NEG, np.float32)
    mask_h = [(_to_bf(tri4), _to_bf(neg4)),              # parity 0: (A, B)
              (_to_bf(zeros4), _to_bf(tri4))]            # parity 1: (A, B)

    in_maps = []
    gmaps = []
    for c in range(8):
        b, h = divmod(c, 2)
        gmap = [2 * j + h for j in range(8)]
        gmaps.append(gmap)
        xb = hs[b]                                        # [2048, 2048]
        xt_h = _to_bf(xb.reshape(NCH, 512, HT, 128).transpose(0, 3, 2, 1))
        xqb = xb.reshape(16, 128, H)[gmap].reshape(NQ, H)  # [1024, 2048]
        xq_h = _to_bf(xqb.reshape(2, 512, HT, 128).transpose(0, 3, 2, 1))
        xtq_h = _to_bf(xqb.reshape(2, 512, HT, 128).transpose(2, 0, 3, 1))
        mA, mB = mask_h[h]
        in_maps.append(dict(
            xt=xt_h, xq=xq_h, xtq=xtq_h,
            wq=wq_l, wk=wk_l, wv=wv_l, wo=wo_l, wfc=wfc_l, wproj=wproj_l,
            bq=bq_r, bk=bk_r, bv=bv_r, bo=bo_r, bfc=bfc_r, bproj=bproj_r,
            maskA=mA, maskB=mB, csq=csq_r, csk=csk_r, csv=csv_r,
        ))

    res = run_bass_kernel_spmd(_get_prog(), in_maps, core_ids=list(range(8)))
    kernel.last_result = res

    out = np.empty((B, S, H), np.float32)
    for c in range(8):
        b, h = divmod(c, 2)
        resT = np.asarray(res.results[c]["out"])          # [2048, 1024]
        blocks = resT.T.reshape(8, 128, H)                # local q-blocks
        for j, g in enumerate(gmaps[c]):
            out[b, g * 128:(g + 1) * 128, :] = blocks[j]
    return out


kernel.last_result = None


# revision 3
# speedup vs baseline: 1.0416x; 1.0324x over previous
"""GPTBigCode transformer block (MQA) on 8 trn2 NeuronCores — v2.

Sharding: data-parallel over batch (4) x parity-interleaved q-block split
(2) per batch element. Core c handles batch c//2 and q-blocks {2j + c%2}.
No collectives; K/V (single MQA head) recomputed per core.

v2 keeps ALL activations feature-on-partition ("T layout") end-to-end —
zero PE transposes. LayerNorm statistics are computed with ones-vector
matmul chains (partition-axis reduction on the tensor engine), per-token
scalars are broadcast back across partitions with K=1 matmuls. Attention
computes transposed scores (keys-on-partition) so softmax-denominators
come from ones-matmuls and probs feed attn@V directly. The softmax
normalization is applied as a per-column multiply on the attention
output. Causal masking of the parity-dependent diagonal zone uses two
per-core mask inputs so the compiled program is identical on all cores.

Weights are host-packed so every weight DMA is contiguous per partition
line; activations never round-trip through DRAM. Matmul inputs bf16;
accumulation, softmax and residual math f32 (residual stream bf16).
"""

import numpy as np
import ml_dtypes

# ---------------------------------------------------------------------------
# Workaround: this container's walrus build rejects >1 sync-wait on
# CTRL-class (Drain) instructions. Split the Tile tail-drain's waits into
# individual wait-carrying NOPs on the SP engine.
import bass_rust
from concourse.tile import TileContext
from concourse.vector_clock import ScopedClock


def _patched_drain_and_barrier(self, tick_clock, wait_clock):
    nc = self.nc
    drain_inst = nc.sync.drain()
    wait_clock.add_sem_waits(
        drain_inst.ins, ScopedClock({None: tick_clock.global_clock})
    )
    si = drain_inst.ins.sync_info
    waits = list(si.on_wait) if si and si.on_wait else []
    if len(waits) > 1:
        drain_inst.ins.sync_info = bass_rust.SyncInfo(
            on_wait=waits[:1],
            on_update=list(si.on_update) if si.on_update else [],
        )
        for w in waits[1:]:
            n = nc.sync.nop(nofuse=True, hint="split_drain_wait")
            n.ins.sync_info = bass_rust.SyncInfo(on_wait=[w], on_update=[])
    nc.all_engine_barrier()
    assert self.sems is not None
    popped = nc._tile_sem_poison_stack.pop()
    assert popped is self._sem_poison
    nc.clear_and_free_semaphores(list(self.sems.allocated().values()))
    nc.all_engine_barrier()


TileContext._drain_and_barrier = _patched_drain_and_barrier


def _split_excess_waits(nc, max_waits=1):
    """Rewrite every instruction carrying more than `max_waits` sem-waits:
    excess waits move onto same-engine NOPs inserted just before it."""
    all_bbs = [bb for fn in nc.m.functions for bb in fn.blocks]
    for bb in all_bbs:
        insts = list(bb.instructions)
        new_list = []
        changed = False
        for inst in insts:
            si = inst.sync_info
            waits = list(si.on_wait) if si and si.on_wait else []
            if len(waits) > max_waits:
                changed = True
                inst.sync_info = bass_rust.SyncInfo(
                    on_wait=waits[:max_waits],
                    on_update=list(si.on_update) if si.on_update else [],
                )
                for w in waits[max_waits:]:
                    nop_bi = nc.engines[inst.engine].nop(
                        nofuse=True, hint="wsplit"
                    )
                    nop = nop_bi.ins
                    cur = nc.cur_bb.bb
                    cl = list(cur.instructions)
                    assert cl and cl[-1].name == nop.name, "nop not appended last"
                    cur.instructions = cl[:-1]
                    nop.sync_info = bass_rust.SyncInfo(on_wait=[w], on_update=[])
                    new_list.append(nop)
            new_list.append(inst)
        if changed:
            bb.instructions = new_list
# ---------------------------------------------------------------------------

import concourse.bass as bass
import concourse.mybir as mybir
from concourse.bass_utils import run_bass_kernel_spmd
from concourse.masks import make_identity

f32 = mybir.dt.float32
bf16 = mybir.dt.bfloat16
AF = mybir.ActivationFunctionType
ALU = mybir.AluOpType

H = 2048
NH = 16
D = 128
INTER = 8192
S = 2048
B = 4
NQ = 1024          # query tokens per core
HT = H // 128      # 16
IT = INTER // 128  # 64
NCH = S // 512     # 4 full-seq chunks
EPS = 1e-5
NEG = -30000.0
INV_H = 1.0 / H


def _ln_rows(nc, rowp, psB, sum_ps, sumsq_ps, eps_t):
    """[1,512] psum sums -> (m_sb f32, rstd_sb f32) row tiles."""
    m_sb = rowp.tile([1, 512], f32, tag="rows", bufs=3, name="m_sb")
    nc.scalar.mul(m_sb, sum_ps, INV_H)
    v_sb = rowp.tile([1, 512], f32, tag="rows", bufs=3, name="v_sb")
    nc.scalar.mul(v_sb, sumsq_ps, INV_H)
    m2 = rowp.tile([1, 512], f32, tag="rows", bufs=3, name="m2")
    nc.vector.tensor_mul(m2, m_sb, m_sb)
    nc.vector.tensor_sub(v_sb, v_sb, m2)
    # rstd = exp(-0.5*ln(var+eps)) — keeps the whole row path on ScalarE
    # (DVE reciprocal on a 1-partition row is ~3.3us serial).
    nc.scalar.activation(v_sb, v_sb, AF.Ln, bias=eps_t)
    rstd = rowp.tile([1, 512], f32, tag="rows", bufs=3, name="rstd")
    nc.scalar.activation(rstd, v_sb, AF.Exp, scale=-0.5)
    return m_sb, rstd


def _ln_chunk_stats(nc, workp, rowp, psB, xc, ones_col, eps_t):
    """LN stats for one [128,16,512] bf16 chunk (raw x, T layout)."""
    sum_ps = psB.tile([1, 512], f32, tag="B", bufs=2, name="sum_ps")
    sumsq_ps = psB.tile([1, 512], f32, tag="B", bufs=2, name="sumsq_ps")
    for kt in range(HT):
        nc.tensor.matmul(sum_ps, ones_col, xc[:, kt, :],
                         start=(kt == 0), stop=(kt == HT - 1))
    for kt in range(HT):
        sq = workp.tile([128, 512], bf16, tag="t2k", bufs=2, name="sq")
        nc.scalar.square(sq, xc[:, kt, :])
        nc.tensor.matmul(sumsq_ps, ones_col, sq,
                         start=(kt == 0), stop=(kt == HT - 1))
    return _ln_rows(nc, rowp, psB, sum_ps, sumsq_ps, eps_t)


def _bcast_row_bf16(nc, workp, psC, ones_rowf, row_sb, name):
    """[1,512] f32 row -> [128,512] bf16 sbuf broadcast tile."""
    bc_ps = psC.tile([128, 512], f32, tag="C", bufs=2, name=f"{name}_ps")
    nc.tensor.matmul(bc_ps, ones_rowf, row_sb, start=True, stop=True)
    bc_sb = workp.tile([128, 512], bf16, tag="bc1k", bufs=2, name=f"{name}_sb")
    nc.scalar.copy(bc_sb, bc_ps)
    return bc_sb


def _scale_inplace(nc, xc, bcr):
    """xc *= rstd (per column). The mean is folded into the following
    matmul chains as a K=1 rank-1 correction with -colsum(W)."""
    for kt in range(HT):
        nc.vector.tensor_mul(xc[:, kt, :], xc[:, kt, :], bcr)


def _build_program():
    nc = bass.Bass(trn_type="TRN2")

    xt_d = nc.dram_tensor("xt", [NCH, 128, HT, 512], bf16, kind="ExternalInput")
    xq_d = nc.dram_tensor("xq", [2, 128, HT, 512], bf16, kind="ExternalInput")
    xtq_d = nc.dram_tensor("xtq", [HT, 2, 128, 512], bf16, kind="ExternalInput")
    wq_d = nc.dram_tensor("wq", [NH, 128, HT, 128], bf16, kind="ExternalInput")
    wk_d = nc.dram_tensor("wk", [128, HT, 128], bf16, kind="ExternalInput")
    wv_d = nc.dram_tensor("wv", [128, HT, 128], bf16, kind="ExternalInput")
    wo_d = nc.dram_tensor("wo", [HT, 128, HT, 128], bf16, kind="ExternalInput")
    wfc_d = nc.dram_tensor("wfc", [IT, 128, HT, 128], bf16, kind="ExternalInput")
    wproj_d = nc.dram_tensor("wproj", [HT, 128, IT, 128], bf16, kind="ExternalInput")
    bq_d = nc.dram_tensor("bq", [128, NH], f32, kind="ExternalInput")
    bk_d = nc.dram_tensor("bk", [128, 1], f32, kind="ExternalInput")
    bv_d = nc.dram_tensor("bv", [1, 128], bf16, kind="ExternalInput")
    bo_d = nc.dram_tensor("bo", [128, HT], f32, kind="ExternalInput")
    bfc_d = nc.dram_tensor("bfc", [128, IT], f32, kind="ExternalInput")
    bproj_d = nc.dram_tensor("bproj", [128, HT], f32, kind="ExternalInput")
    maskA_d = nc.dram_tensor("maskA", [128, 512], bf16, kind="ExternalInput")
    maskB_d = nc.dram_tensor("maskB", [128, 512], bf16, kind="ExternalInput")
    csq_d = nc.dram_tensor("csq", [1, NH, 128], bf16, kind="ExternalInput")
    csk_d = nc.dram_tensor("csk", [1, 128], bf16, kind="ExternalInput")
    csv_d = nc.dram_tensor("csv", [1, 128], bf16, kind="ExternalInput")
    out_d = nc.dram_tensor("out", [H, NQ], f32, kind="ExternalOutput")

    with TileContext(nc) as tc:
        with (
            tc.tile_pool(name="const", bufs=1) as constp,
            tc.tile_pool(name="big", bufs=1) as bigp,
            tc.tile_pool(name="s32", bufs=2) as s32p,
            tc.tile_pool(name="str16", bufs=2) as strp,
            tc.tile_pool(name="band", bufs=3) as bandp,
            tc.tile_pool(name="work", bufs=2) as workp,
            tc.tile_pool(name="rows", bufs=4) as rowp,
            tc.tile_pool(name="psA", bufs=2, space="PSUM") as psA,
            tc.tile_pool(name="psB", bufs=2, space="PSUM") as psB,
            tc.tile_pool(name="psC", bufs=2, space="PSUM") as psC,
        ):
            # First input chunk DMA goes out before the constant loads so
            # compute can start as early as possible.
            xc0 = strp.tile([128, HT, 512], bf16, tag="str16", name="xc")
            nc.sync.dma_start(xc0[:, 0:8, :], xt_d[0][:, 0:8, :])
            nc.sync.dma_start(xc0[:, 8:16, :], xt_d[0][:, 8:16, :])

            # ---- constants ----
            ones_col = constp.tile([128, 1], bf16, name="ones_col")
            nc.vector.memset(ones_col, 1.0)
            ones_rowf = constp.tile([1, 128], f32, name="ones_rowf")
            nc.vector.memset(ones_rowf, 1.0)
            ones_rowb = constp.tile([1, 128], bf16, name="ones_rowb")
            nc.vector.memset(ones_rowb, 1.0)
            eps_t = constp.tile([1, 1], f32, name="eps_t")
            nc.vector.memset(eps_t, EPS)
            bq_sb = constp.tile([128, NH], f32, name="bq_sb")
            nc.sync.dma_start(bq_sb, bq_d[:, :])
            bk_sb = constp.tile([128, 1], f32, name="bk_sb")
            nc.sync.dma_start(bk_sb, bk_d[:, :])
            bv_sb = constp.tile([1, 128], bf16, name="bv_sb")
            nc.sync.dma_start(bv_sb, bv_d[:, :])
            bo_sb = constp.tile([128, HT], f32, name="bo_sb")
            nc.sync.dma_start(bo_sb, bo_d[:, :])
            bfc_sb = constp.tile([128, IT], f32, name="bfc_sb")
            nc.sync.dma_start(bfc_sb, bfc_d[:, :])
            bproj_sb = constp.tile([128, HT], f32, name="bproj_sb")
            nc.sync.dma_start(bproj_sb, bproj_d[:, :])
            maskA = constp.tile([128, 512], bf16, name="maskA")
            nc.sync.dma_start(maskA, maskA_d[:, :])
            maskB = constp.tile([128, 512], bf16, name="maskB")
            nc.sync.dma_start(maskB, maskB_d[:, :])
            csq_sb = constp.tile([1, NH, 128], bf16, name="csq_sb")
            nc.sync.dma_start(csq_sb, csq_d[:, :, :])
            csk_sb = constp.tile([1, 128], bf16, name="csk_sb")
            nc.sync.dma_start(csk_sb, csk_d[:, :])
            csv_sb = constp.tile([1, 128], bf16, name="csv_sb")
            nc.sync.dma_start(csv_sb, csv_d[:, :])
            id_bf = constp.tile([128, 128], bf16, name="id_bf")
            make_identity(nc, id_bf)
            kT_sb = constp.tile([128, S], bf16, name="kT_sb")
            vtm = constp.tile([128, HT, 128], bf16, name="vtm")

            # ---- phase A: full-seq LN1 + K/V, streamed in 512-token chunks
            for ci in range(NCH):
                if ci == 0:
                    xc = xc0
                else:
                    xc = strp.tile([128, HT, 512], bf16, tag="str16", name="xc")
                    nc.sync.dma_start(xc, xt_d[ci])
                wk_sb = bandp.tile([128, HT, 128], bf16, tag="band4", bufs=3,
                                   name="wk_sb")
                nc.sync.dma_start(wk_sb, wk_d[:, :, :])
                wv_sb = bandp.tile([128, HT, 128], bf16, tag="band4", bufs=3,
                                   name="wv_sb")
                nc.sync.dma_start(wv_sb, wv_d[:, :, :])
                m_sb, rstd = _ln_chunk_stats(nc, workp, rowp, psB, xc,
                                             ones_col, eps_t)
                bcr = _bcast_row_bf16(nc, workp, psC, ones_rowf, rstd, "bcr")
                mr = workp.tile([1, 512], bf16, tag="xq1k", bufs=2, name="mr")
                nc.vector.tensor_mul(mr, m_sb, rstd)
                _scale_inplace(nc, xc, bcr)
                # K^T chunk: [dk=128, 512 tokens]
                kps = psA.tile([128, 2, 512], f32, tag="A", bufs=2, name="kps")
                for kt in range(HT):
                    nc.tensor.matmul(kps[:, 0, :], wk_sb[:, kt, :], xc[:, kt, :],
                                     start=(kt == 0), stop=False)
                nc.tensor.matmul(kps[:, 0, :], csk_sb, mr,
                                 start=False, stop=True)
                nc.scalar.activation(kT_sb[:, ci * 512:(ci + 1) * 512],
                                     kps[:, 0, :], AF.Identity, bias=bk_sb)
                # V token-major: 4 token-blocks
                for tb in range(4):
                    vps = psA.tile([128, 2, 512], f32, tag="A", bufs=2, name="vps")
                    for kt in range(HT):
                        nc.tensor.matmul(
                            vps[:, 0, 0:128],
                            xc[:, kt, tb * 128:(tb + 1) * 128],
                            wv_sb[:, kt, :], start=(kt == 0), stop=False,
                        )
                    nc.tensor.matmul(vps[:, 0, 0:128], ones_rowb, bv_sb,
                                     start=False, stop=False)
                    nc.tensor.matmul(vps[:, 0, 0:128],
                                     mr[:, tb * 128:(tb + 1) * 128], csv_sb,
                                     start=False, stop=True)
                    nc.vector.tensor_copy(vtm[:, ci * 4 + tb, :], vps[:, 0, 0:128])

            # ---- phase B: own-q LN1 + Q projection (2 chunks of 512) ----
            # qT layout [dq, hg, j, hh, q]: scores rhs [:, hg, j] is a fully
            # contiguous 512-column block (4 heads x 128 q).
            qT = s32p.tile([128, 4, 8, 4, 128], bf16, tag="s32", name="qT")
            for ch in range(2):
                xqc = strp.tile([128, HT, 512], bf16, tag="str16", name="xqc")
                nc.sync.dma_start(xqc, xq_d[ch])
                m_sb, rstd = _ln_chunk_stats(nc, workp, rowp, psB, xqc,
                                             ones_col, eps_t)
                bcr = _bcast_row_bf16(nc, workp, psC, ones_rowf, rstd, "qbcr")
                mr = workp.tile([1, 512], bf16, tag="xq1k", bufs=2, name="qmr")
                nc.vector.tensor_mul(mr, m_sb, rstd)
                _scale_inplace(nc, xqc, bcr)
                for m in range(NH):
                    hg, hh = divmod(m, 4)
                    band = bandp.tile([128, HT, 128], bf16, tag="band4",
                                      bufs=3, name="band")
                    nc.sync.dma_start(band, wq_d[m])
                    qps = psA.tile([128, 2, 512], f32, tag="A", bufs=2, name="qps")
                    for kt in range(HT):
                        nc.tensor.matmul(qps[:, 0, :], band[:, kt, :],
                                         xqc[:, kt, :],
                                         start=(kt == 0), stop=False)
                    nc.tensor.matmul(qps[:, 0, :], csq_sb[:, m, :], mr,
                                     start=False, stop=True)
                    nc.scalar.activation(qT[:, hg, 4 * ch:4 * ch + 4, hh, :],
                                         qps[:, 0, :], AF.Identity,
                                         bias=bq_sb[:, m:m + 1])

            # ---- phase C: attention (scoresT, padded extent E=2j+2) ----
            # Two q-blocks' ladders are interleaved per head-group so the PE
            # fills the ACT-exp latency of one block with the other block's
            # matmuls.
            attnT = s32p.tile([128, NH, NQ], bf16, tag="s32", name="attnT")

            def _attn_step(hg, j, p0, kts, den_ps, av_ps):
                """One 2-kt step of block j: two scores matmuls into a 2-bank
                psum tile, one exp over both, then den/av accumulation."""
                E = len(kts)
                sc = psA.tile([128, 2, 512], f32, tag="A", bufs=2, name="sc")
                for dk in range(2):
                    kt = kts[p0 + dk]
                    masked = kt >= E - 2
                    nc.tensor.matmul(
                        sc[:, dk, :], kT_sb[:, kt * 128:(kt + 1) * 128],
                        qT[:, hg, j], start=True, stop=not masked,
                    )
                    if masked:
                        # mask add on the PE: sc += I^T @ mask
                        nc.tensor.matmul(sc[:, dk, :], id_bf,
                                         maskA if kt == E - 2 else maskB,
                                         start=False, stop=True)
                ex = workp.tile([128, 2, 512], bf16, tag="t2k", bufs=2,
                                name="ex")
                nc.scalar.activation(ex, sc, AF.Exp)
                for dk in range(2):
                    idx = p0 + dk
                    kt = kts[idx]
                    nc.tensor.matmul(den_ps, ones_col, ex[:, dk, :],
                                     start=(idx == 0), stop=(idx == E - 1))
                    nc.tensor.matmul(av_ps, vtm[:, kt, :], ex[:, dk, :],
                                     start=(idx == 0), stop=(idx == E - 1))

            def _attn_tail(hg, j, den_ps, av_ps):
                lnd = rowp.tile([1, 512], f32, tag="rows", bufs=3, name="lnd")
                nc.scalar.activation(lnd, den_ps, AF.Ln)
                rec = rowp.tile([1, 512], f32, tag="rows", bufs=3, name="rec")
                nc.scalar.activation(rec, lnd, AF.Exp, scale=-1.0)
                bcr_ps = psB.tile([128, 512], f32, tag="B", bufs=2,
                                  name="bcr_ps")
                nc.tensor.matmul(bcr_ps, ones_rowf, rec, start=True, stop=True)
                bcr_sb = workp.tile([128, 512], bf16, tag="bc1k", bufs=2,
                                    name="bcr_sb")
                nc.scalar.copy(bcr_sb, bcr_ps)
                nc.vector.tensor_mul(
                    attnT[:, hg * 4:(hg + 1) * 4, j * 128:(j + 1) * 128],
                    av_ps, bcr_sb,
                )

            for hg in range(4):
                for jp in range(4):
                    j0, j1 = 2 * jp, 2 * jp + 1
                    E0, E1 = 2 * j0 + 2, 2 * j1 + 2
                    kts0 = [E0 - 2, E0 - 1] + list(range(E0 - 2))
                    kts1 = [E1 - 2, E1 - 1] + list(range(E1 - 2))
                    den0 = psB.tile([1, 512], f32, tag="B", bufs=2, name="den0")
                    av0 = psC.tile([128, 512], f32, tag="C", bufs=2, name="av0")
                    den1 = psB.tile([1, 512], f32, tag="B", bufs=2, name="den1")
                    av1 = psC.tile([128, 512], f32, tag="C", bufs=2, name="av1")
                    for p0 in range(0, E1, 2):
                        if p0 < E0:
                            _attn_step(hg, j0, p0, kts0, den0, av0)
                        elif p0 == E0:
                            _attn_tail(hg, j0, den0, av0)
                        _attn_step(hg, j1, p0, kts1, den1, av1)
                    _attn_tail(hg, j1, den1, av1)

            # ---- phase D: out-proj + residual -> y ----
            # ch-outer: y[:, :, ch0] completes early so LN2+fc of chunk 0
            # overlap the second wo half.
            y = s32p.tile([128, HT, NQ], bf16, tag="s32", name="y")
            for ch in range(2):
                for ob in range(HT):
                    band = bandp.tile([128, HT, 128], bf16, tag="band4", bufs=3,
                                      name="band")
                    nc.sync.dma_start(band, wo_d[ob])
                    wps = psA.tile([128, 2, 512], f32, tag="A", bufs=2,
                                   name="wps")
                    for ht in range(HT):
                        nc.tensor.matmul(wps[:, 0, :], band[:, ht, :],
                                         attnT[:, ht, ch * 512:(ch + 1) * 512],
                                         start=(ht == 0), stop=(ht == HT - 1))
                    xqt = workp.tile([128, 512], bf16, tag="xq1k", bufs=2,
                                     name="xqt")
                    nc.sync.dma_start(xqt, xtq_d[ob, ch])
                    nc.vector.scalar_tensor_tensor(
                        out=y[:, ob, ch * 512:(ch + 1) * 512],
                        in0=wps[:, 0, :], scalar=bo_sb[:, ob:ob + 1],
                        in1=xqt, op0=ALU.add, op1=ALU.add,
                    )

            # ---- phase E: LN2 + MLP + residual -> out (per 512-token chunk)
            for ch in range(2):
                cols = slice(ch * 512, (ch + 1) * 512)
                sum_ps = psB.tile([1, 512], f32, tag="B", bufs=2, name="l2sum")
                sumsq_ps = psB.tile([1, 512], f32, tag="B", bufs=2, name="l2sq")
                for kt in range(HT):
                    nc.tensor.matmul(sum_ps, ones_col, y[:, kt, cols],
                                     start=(kt == 0), stop=(kt == HT - 1))
                for kt in range(HT):
                    sq = workp.tile([128, 512], bf16, tag="t2k", bufs=2,
                                    name="sq2")
                    nc.scalar.square(sq, y[:, kt, cols])
                    nc.tensor.matmul(sumsq_ps, ones_col, sq,
                                     start=(kt == 0), stop=(kt == HT - 1))
                m_sb, rstd = _ln_rows(nc, rowp, psB, sum_ps, sumsq_ps, eps_t)
                bcm = _bcast_row_bf16(nc, workp, psC, ones_rowf, m_sb, "l2bcm")
                bcr = _bcast_row_bf16(nc, workp, psC, ones_rowf, rstd, "l2bcr")
                ln2s = s32p.tile([128, HT, 512], bf16, tag="s32", name="ln2s")
                for kt in range(HT):
                    nc.vector.tensor_sub(ln2s[:, kt, :], y[:, kt, cols], bcm)
                    nc.vector.tensor_mul(ln2s[:, kt, :], ln2s[:, kt, :], bcr)

                gT = bigp.tile([128, IT, 512], bf16, tag="big", name="gT")
                for mb in range(IT):
                    band = bandp.tile([128, HT, 128], bf16, tag="band4",
                                      bufs=3, name="band")
                    nc.sync.dma_start(band, wfc_d[mb])
                    fps = psA.tile([128, 2, 512], f32, tag="A", bufs=2,
                                   name="fps")
                    for kt in range(HT):
                        nc.tensor.matmul(fps[:, 0, :], band[:, kt, :],
                                         ln2s[:, kt, :],
                                         start=(kt == 0), stop=(kt == HT - 1))
                    nc.scalar.activation(gT[:, mb, :], fps[:, 0, :],
                                         AF.Gelu_apprx_tanh,
                                         bias=bfc_sb[:, mb:mb + 1])

                for ob in range(HT):
                    pband = strp.tile([128, IT, 128], bf16, tag="str16",
                                      name="pband")
                    nc.sync.dma_start(pband, wproj_d[ob])
                    pps = psA.tile([128, 2, 512], f32, tag="A", bufs=2,
                                   name="pps")
                    for mt in range(IT):
                        nc.tensor.matmul(pps[:, 0, :], pband[:, mt, :],
                                         gT[:, mt, :],
                                         start=(mt == 0), stop=(mt == IT - 1))
                    osb = workp.tile([128, 512], f32, tag="f2k", bufs=2,
                                     name="osb")
                    nc.vector.scalar_tensor_tensor(
                        out=osb, in0=pps[:, 0, :],
                        scalar=bproj_sb[:, ob:ob + 1],
                        in1=y[:, ob, cols], op0=ALU.add, op1=ALU.add,
                    )
                    nc.sync.dma_start(
                        out_d[ob * 128:(ob + 1) * 128, cols], osb
                    )
    _split_excess_waits(nc)
    return nc


_PROG = None


def _get_prog():
    global _PROG
    if _PROG is None:
        _PROG = _build_program()
    return _PROG


def _to_bf(a):
    return np.ascontiguousarray(a.astype(ml_dtypes.bfloat16))


def kernel(hidden_states, ln1_g, ln1_b, ln2_g, ln2_b, wq, bq, wkv, bkv,
           wo, bo, wfc, bfc, wproj, bproj):
    hs = np.asarray(hidden_states, np.float32)
    ln1_g = np.asarray(ln1_g, np.float32)
    ln1_b = np.asarray(ln1_b, np.float32)
    ln2_g = np.asarray(ln2_g, np.float32)
    ln2_b = np.asarray(ln2_b, np.float32)
    wq = np.asarray(wq, np.float32)
    wkv = np.asarray(wkv, np.float32)
    wo = np.asarray(wo, np.float32)
    wfc = np.asarray(wfc, np.float32)
    wproj = np.asarray(wproj, np.float32)

    # Fold LN gains into the following matmuls; fold qk scale into K.
    wq_e = ln1_g[:, None] * wq
    bq_e = np.asarray(bq, np.float32) + ln1_b @ wq
    wkv_e = ln1_g[:, None] * wkv
    bkv_e = np.asarray(bkv, np.float32) + ln1_b @ wkv
    scale = 1.0 / np.sqrt(D)
    wk_e = wkv_e[:, :D] * scale
    bk_e = bkv_e[:D] * scale
    wv_e = wkv_e[:, D:]
    bv_e = bkv_e[D:]
    wfc_e = ln2_g[:, None] * wfc
    bfc_e = np.asarray(bfc, np.float32) + ln2_b @ wfc

    # Host-packed weight layouts: [out-block, partition, k-tile, n] so each
    # band DMA is contiguous per partition line.
    wq_l = _to_bf(wq_e.reshape(HT, 128, NH, 128).transpose(2, 1, 0, 3))
    wk_l = _to_bf(wk_e.reshape(HT, 128, 128).transpose(1, 0, 2))
    wv_l = _to_bf(wv_e.reshape(HT, 128, 128).transpose(1, 0, 2))
    wo_l = _to_bf(wo.reshape(HT, 128, HT, 128).transpose(2, 1, 0, 3))
    wfc_l = _to_bf(wfc_e.reshape(HT, 128, IT, 128).transpose(2, 1, 0, 3))
    wproj_l = _to_bf(wproj.reshape(IT, 128, HT, 128).transpose(2, 1, 0, 3))

    # Negated column sums for the K=1 LN-mean-fold correction matmuls.
    csq_r = _to_bf(-wq_e.sum(axis=0).reshape(1, NH, 128))
    csk_r = _to_bf(-wk_e.sum(axis=0)[None, :])
    csv_r = _to_bf(-wv_e.sum(axis=0)[None, :])

    bq_r = np.ascontiguousarray(bq_e.reshape(NH, 128).T)
    bk_r = np.ascontiguousarray(bk_e[:, None])
    bv_r = _to_bf(bv_e[None, :])
    bo_r = np.ascontiguousarray(np.asarray(bo, np.float32).reshape(HT, 128).T)
    bfc_r = np.ascontiguousarray(bfc_e.reshape(IT, 128).T)
    bproj_r = np.ascontiguousarray(
        np.asarray(bproj, np.float32).reshape(HT, 128).T)

    # Causal masks for the two parity-dependent diagonal k-tiles.
    tri = np.where(np.arange(128)[None, :] >= np.arange(128)[:, None],
                   0.0, NEG).astype(np.float32)          # [k,q]
    tri4 = np.tile(tri, (1, 4))                          # [128, 512] (4 heads)
    zeros4 = np.zeros((128, 512), np.float32)
    neg4 = np.full((128, 512), NEG, np.float32)
    mask_h = [(_to_bf(tri4), _to_bf(neg4)),              # parity 0: (A, B)
              (_to_bf(zeros4), _to_bf(tri4))]            # parity 1: (A, B)

    in_maps = []
    gmaps = []
    for c in range(8):
        b, h = divmod(c, 2)
        gmap = [2 * j + h for j in range(8)]
        gmaps.append(gmap)
        xb = hs[b]                                        # [2048, 2048]
        xt_h = _to_bf(xb.reshape(NCH, 512, HT, 128).transpose(0, 3, 2, 1))
        xqb = xb.reshape(16, 128, H)[gmap].reshape(NQ, H)  # [1024, 2048]
        xq_h = _to_bf(xqb.reshape(2, 512, HT, 128).transpose(0, 3, 2, 1))
        xtq_h = _to_bf(xqb.reshape(2, 512, HT, 128).transpose(2, 0, 3, 1))
        mA, mB = mask_h[h]
        in_maps.append(dict(
            xt=xt_h, xq=xq_h, xtq=xtq_h,
            wq=wq_l, wk=wk_l, wv=wv_l, wo=wo_l, wfc=wfc_l, wproj=wproj_l,
            bq=bq_r, bk=bk_r, bv=bv_r, bo=bo_r, bfc=bfc_r, bproj=bproj_r,
            maskA=mA, maskB=mB, csq=csq_r, csk=csk_r, csv=csv_r,
        ))

    res = run_bass_kernel_spmd(_get_prog(), in_maps, core_ids=list(range(8)))
    kernel.last_result = res

    out = np.empty((B, S, H), np.float32)
    for c in range(8):
        b, h = divmod(c, 2)
        resT = np.asarray(res.results[c]["out"])          # [2048, 1024]
        blocks = resT.T.reshape(8, 128, H)                # local q-blocks
        for j, g in enumerate(gmaps[c]):
            out[b, g * 128:(g + 1) * 128, :] = blocks[j]
    return out


kernel.last_result = None


# revision 4
# speedup vs baseline: 1.0534x; 1.0113x over previous
"""GPTBigCode transformer block (MQA) on 8 trn2 NeuronCores — v2.

Sharding: data-parallel over batch (4) x parity-interleaved q-block split
(2) per batch element. Core c handles batch c//2 and q-blocks {2j + c%2}.
No collectives; K/V (single MQA head) recomputed per core.

v2 keeps ALL activations feature-on-partition ("T layout") end-to-end —
zero PE transposes. LayerNorm statistics are computed with ones-vector
matmul chains (partition-axis reduction on the tensor engine), per-token
scalars are broadcast back across partitions with K=1 matmuls. Attention
computes transposed scores (keys-on-partition) so softmax-denominators
come from ones-matmuls and probs feed attn@V directly. The softmax
normalization is applied as a per-column multiply on the attention
output. Causal masking of the parity-dependent diagonal zone uses two
per-core mask inputs so the compiled program is identical on all cores.

Weights are host-packed so every weight DMA is contiguous per partition
line; activations never round-trip through DRAM. Matmul inputs bf16;
accumulation, softmax and residual math f32 (residual stream bf16).
"""

import numpy as np
import ml_dtypes

# ---------------------------------------------------------------------------
# Workaround: this container's walrus build rejects >1 sync-wait on
# CTRL-class (Drain) instructions. Split the Tile tail-drain's waits into
# individual wait-carrying NOPs on the SP engine.
import bass_rust
from concourse.tile import TileContext
from concourse.vector_clock import ScopedClock


def _patched_drain_and_barrier(self, tick_clock, wait_clock):
    nc = self.nc
    drain_inst = nc.sync.drain()
    wait_clock.add_sem_waits(
        drain_inst.ins, ScopedClock({None: tick_clock.global_clock})
    )
    si = drain_inst.ins.sync_info
    waits = list(si.on_wait) if si and si.on_wait else []
    if len(waits) > 1:
        drain_inst.ins.sync_info = bass_rust.SyncInfo(
            on_wait=waits[:1],
            on_update=list(si.on_update) if si.on_update else [],
        )
        for w in waits[1:]:
            n = nc.sync.nop(nofuse=True, hint="split_drain_wait")
            n.ins.sync_info = bass_rust.SyncInfo(on_wait=[w], on_update=[])
    nc.all_engine_barrier()
    assert self.sems is not None
    popped = nc._tile_sem_poison_stack.pop()
    assert popped is self._sem_poison
    nc.clear_and_free_semaphores(list(self.sems.allocated().values()))
    nc.all_engine_barrier()


TileContext._drain_and_barrier = _patched_drain_and_barrier


def _split_excess_waits(nc, max_waits=1):
    """Rewrite every instruction carrying more than `max_waits` sem-waits:
    excess waits move onto same-engine NOPs inserted just before it."""
    all_bbs = [bb for fn in nc.m.functions for bb in fn.blocks]
    for bb in all_bbs:
        insts = list(bb.instructions)
        new_list = []
        changed = False
        for inst in insts:
            si = inst.sync_info
            waits = list(si.on_wait) if si and si.on_wait else []
            if len(waits) > max_waits:
                changed = True
                inst.sync_info = bass_rust.SyncInfo(
                    on_wait=waits[:max_waits],
                    on_update=list(si.on_update) if si.on_update else [],
                )
                for w in waits[max_waits:]:
                    nop_bi = nc.engines[inst.engine].nop(
                        nofuse=True, hint="wsplit"
                    )
                    nop = nop_bi.ins
                    cur = nc.cur_bb.bb
                    cl = list(cur.instructions)
                    assert cl and cl[-1].name == nop.name, "nop not appended last"
                    cur.instructions = cl[:-1]
                    nop.sync_info = bass_rust.SyncInfo(on_wait=[w], on_update=[])
                    new_list.append(nop)
            new_list.append(inst)
        if changed:
            bb.instructions = new_list
# ---------------------------------------------------------------------------

import concourse.bass as bass
import concourse.mybir as mybir
from concourse.bass_utils import run_bass_kernel_spmd
from concourse.masks import make_identity

f32 = mybir.dt.float32
bf16 = mybir.dt.bfloat16
AF = mybir.ActivationFunctionType
ALU = mybir.AluOpType

H = 2048
NH = 16
D = 128
INTER = 8192
S = 2048
B = 4
NQ = 1024          # query tokens per core
HT = H // 128      # 16
IT = INTER // 128  # 64
NCH = S // 512     # 4 full-seq chunks
EPS = 1e-5
NEG = -30000.0
INV_H = 1.0 / H


def _ln_rows(nc, rowp, workp2, psB, sum_ps, sumsq_ps, eps_t):
    """[1,512] psum sums -> (m_sb f32, rstd_sb f32) row tiles."""
    m_sb = rowp.tile([1, 512], f32, tag="rows", bufs=2, name="m_sb")
    nc.scalar.mul(m_sb, sum_ps, INV_H)
    v_sb = rowp.tile([1, 512], f32, tag="rows", bufs=2, name="v_sb")
    nc.scalar.mul(v_sb, sumsq_ps, INV_H)
    m2 = workp2.tile([128, 512], f32, tag="t2k", bufs=2, name="m2")
    nc.vector.tensor_mul(m2[0:1, :], m_sb, m_sb)
    nc.vector.tensor_sub(v_sb, v_sb, m2[0:1, :])
    # rstd = exp(-0.5*ln(var+eps)) — keeps the whole row path on ScalarE
    # (DVE reciprocal on a 1-partition row is ~3.3us serial); both steps
    # in place so the rows tag needs only 2 slots.
    nc.scalar.activation(v_sb, v_sb, AF.Ln, bias=eps_t)
    nc.scalar.activation(v_sb, v_sb, AF.Exp, scale=-0.5)
    return m_sb, v_sb


def _ln_chunk_stats(nc, workp, rowp, psB, psC, xc, ones_col, eps_t):
    """LN stats for one [128,16,512] bf16 chunk (raw x, T layout). sum and
    sumsq live in different psum tags so consecutive chunks' stats chains
    double-buffer instead of serializing on one tag pair."""
    sum_ps = psB.tile([1, 512], f32, tag="B", bufs=2, name="sum_ps")
    sumsq_ps = psC.tile([1, 512], f32, tag="C", bufs=2, name="sumsq_ps")
    for kt in range(HT):
        nc.tensor.matmul(sum_ps, ones_col, xc[:, kt, :],
                         start=(kt == 0), stop=(kt == HT - 1))
    for kt in range(HT):
        sq = workp.tile([128, 512], bf16, tag="t2k", bufs=2, name="sq")
        nc.scalar.square(sq, xc[:, kt, :])
        nc.tensor.matmul(sumsq_ps, ones_col, sq,
                         start=(kt == 0), stop=(kt == HT - 1))
    return _ln_rows(nc, rowp, workp, psB, sum_ps, sumsq_ps, eps_t)


def _bcast_row_bf16(nc, workp, psC, ones_rowf, row_sb, name):
    """[1,512] f32 row -> [128,512] bf16 sbuf broadcast tile."""
    bc_ps = psC.tile([128, 512], f32, tag="C", bufs=2, name=f"{name}_ps")
    nc.tensor.matmul(bc_ps, ones_rowf, row_sb, start=True, stop=True)
    bc_sb = workp.tile([128, 512], bf16, tag="bc1k", bufs=2, name=f"{name}_sb")
    nc.scalar.copy(bc_sb, bc_ps)
    return bc_sb


def _scale_inplace(nc, xc, bcr):
    """xc *= rstd (per column). The mean is folded into the following
    matmul chains as a K=1 rank-1 correction with -colsum(W)."""
    for kt in range(HT):
        nc.vector.tensor_mul(xc[:, kt, :], xc[:, kt, :], bcr)


def _build_program():
    nc = bass.Bass(trn_type="TRN2")

    xt_d = nc.dram_tensor("xt", [NCH, 128, HT, 512], bf16, kind="ExternalInput")
    xq_d = nc.dram_tensor("xq", [2, 128, HT, 512], bf16, kind="ExternalInput")
    xtq_d = nc.dram_tensor("xtq", [HT, 2, 128, 512], bf16, kind="ExternalInput")
    wq_d = nc.dram_tensor("wq", [NH, 128, HT, 128], bf16, kind="ExternalInput")
    wk_d = nc.dram_tensor("wk", [128, HT, 128], bf16, kind="ExternalInput")
    wv_d = nc.dram_tensor("wv", [128, HT, 128], bf16, kind="ExternalInput")
    wo_d = nc.dram_tensor("wo", [HT, 128, HT, 128], bf16, kind="ExternalInput")
    wfc_d = nc.dram_tensor("wfc", [IT, 128, HT, 128], bf16, kind="ExternalInput")
    wproj_d = nc.dram_tensor("wproj", [HT, 128, IT, 128], bf16, kind="ExternalInput")
    bq_d = nc.dram_tensor("bq", [128, NH], f32, kind="ExternalInput")
    bk_d = nc.dram_tensor("bk", [128, 1], f32, kind="ExternalInput")
    bv_d = nc.dram_tensor("bv", [1, 128], bf16, kind="ExternalInput")
    bo_d = nc.dram_tensor("bo", [128, HT], f32, kind="ExternalInput")
    bfc_d = nc.dram_tensor("bfc", [128, IT], f32, kind="ExternalInput")
    bproj_d = nc.dram_tensor("bproj", [128, HT], f32, kind="ExternalInput")
    maskA_d = nc.dram_tensor("maskA", [128, 512], bf16, kind="ExternalInput")
    maskB_d = nc.dram_tensor("maskB", [128, 512], bf16, kind="ExternalInput")
    csq_d = nc.dram_tensor("csq", [1, NH, 128], bf16, kind="ExternalInput")
    csk_d = nc.dram_tensor("csk", [1, 128], bf16, kind="ExternalInput")
    csv_d = nc.dram_tensor("csv", [1, 128], bf16, kind="ExternalInput")
    out_d = nc.dram_tensor("out", [H, NQ], f32, kind="ExternalOutput")

    with TileContext(nc) as tc:
        with (
            tc.tile_pool(name="const", bufs=1) as constp,
            tc.tile_pool(name="big", bufs=1) as bigp,
            tc.tile_pool(name="s32", bufs=2) as s32p,
            tc.tile_pool(name="str16", bufs=2) as strp,
            tc.tile_pool(name="band", bufs=3) as bandp,
            tc.tile_pool(name="work", bufs=2) as workp,
            tc.tile_pool(name="rows", bufs=4) as rowp,
            tc.tile_pool(name="psA", bufs=2, space="PSUM") as psA,
            tc.tile_pool(name="psB", bufs=2, space="PSUM") as psB,
            tc.tile_pool(name="psC", bufs=2, space="PSUM") as psC,
        ):
            # First input chunk DMA goes out before the constant loads so
            # compute can start as early as possible.
            xc0 = strp.tile([128, HT, 512], bf16, tag="str16", name="xc")
            nc.sync.dma_start(xc0[:, 0:8, :], xt_d[0][:, 0:8, :])
            nc.sync.dma_start(xc0[:, 8:16, :], xt_d[0][:, 8:16, :])

            # ---- constants ----
            ones_col = constp.tile([128, 1], bf16, name="ones_col")
            nc.vector.memset(ones_col, 1.0)
            ones_rowf = constp.tile([1, 128], f32, name="ones_rowf")
            nc.vector.memset(ones_rowf, 1.0)
            ones_rowb = constp.tile([1, 128], bf16, name="ones_rowb")
            nc.vector.memset(ones_rowb, 1.0)
            eps_t = constp.tile([1, 1], f32, name="eps_t")
            nc.vector.memset(eps_t, EPS)
            bq_sb = constp.tile([128, NH], f32, name="bq_sb")
            nc.sync.dma_start(bq_sb, bq_d[:, :])
            bk_sb = constp.tile([128, 1], f32, name="bk_sb")
            nc.sync.dma_start(bk_sb, bk_d[:, :])
            bv_sb = constp.tile([1, 128], bf16, name="bv_sb")
            nc.sync.dma_start(bv_sb, bv_d[:, :])
            bo_sb = constp.tile([128, HT], f32, name="bo_sb")
            nc.sync.dma_start(bo_sb, bo_d[:, :])
            bfc_sb = constp.tile([128, IT], f32, name="bfc_sb")
            nc.sync.dma_start(bfc_sb, bfc_d[:, :])
            bproj_sb = constp.tile([128, HT], f32, name="bproj_sb")
            nc.sync.dma_start(bproj_sb, bproj_d[:, :])
            maskA = constp.tile([128, 512], bf16, name="maskA")
            nc.sync.dma_start(maskA, maskA_d[:, :])
            maskB = constp.tile([128, 512], bf16, name="maskB")
            nc.sync.dma_start(maskB, maskB_d[:, :])
            csq_sb = constp.tile([1, NH, 128], bf16, name="csq_sb")
            nc.sync.dma_start(csq_sb, csq_d[:, :, :])
            csk_sb = constp.tile([1, 128], bf16, name="csk_sb")
            nc.sync.dma_start(csk_sb, csk_d[:, :])
            csv_sb = constp.tile([1, 128], bf16, name="csv_sb")
            nc.sync.dma_start(csv_sb, csv_d[:, :])
            id_bf = constp.tile([128, 128], bf16, name="id_bf")
            make_identity(nc, id_bf)
            kT_sb = constp.tile([128, S], bf16, name="kT_sb")
            vtm = constp.tile([128, HT, 128], bf16, name="vtm")

            # ---- phase A: full-seq LN1 + K/V, streamed in 512-token chunks
            for ci in range(NCH):
                if ci == 0:
                    xc = xc0
                else:
                    xc = strp.tile([128, HT, 512], bf16, tag="str16", name="xc")
                    nc.sync.dma_start(xc, xt_d[ci])
                wk_sb = bandp.tile([128, HT, 128], bf16, tag="band4", bufs=3,
                                   name="wk_sb")
                nc.sync.dma_start(wk_sb, wk_d[:, :, :])
                wv_sb = bandp.tile([128, HT, 128], bf16, tag="band4", bufs=3,
                                   name="wv_sb")
                nc.sync.dma_start(wv_sb, wv_d[:, :, :])
                m_sb, rstd = _ln_chunk_stats(nc, workp, rowp, psB, psC, xc,
                                             ones_col, eps_t)
                bcr = _bcast_row_bf16(nc, workp, psC, ones_rowf, rstd, "bcr")
                mr = workp.tile([1, 512], bf16, tag="xq1k", bufs=2, name="mr")
                nc.vector.tensor_mul(mr, m_sb, rstd)
                _scale_inplace(nc, xc, bcr)
                # K^T chunk: [dk=128, 512 tokens]
                kps = psA.tile([128, 2, 512], f32, tag="A", bufs=2, name="kps")
                for kt in range(HT):
                    nc.tensor.matmul(kps[:, 0, :], wk_sb[:, kt, :], xc[:, kt, :],
                                     start=(kt == 0), stop=False)
                nc.tensor.matmul(kps[:, 0, :], csk_sb, mr,
                                 start=False, stop=True)
                nc.scalar.activation(kT_sb[:, ci * 512:(ci + 1) * 512],
                                     kps[:, 0, :], AF.Identity, bias=bk_sb)
                # V token-major: 4 token-blocks
                for tb in range(4):
                    vps = psA.tile([128, 2, 512], f32, tag="A", bufs=2, name="vps")
                    for kt in range(HT):
                        nc.tensor.matmul(
                            vps[:, 0, 0:128],
                            xc[:, kt, tb * 128:(tb + 1) * 128],
                            wv_sb[:, kt, :], start=(kt == 0), stop=False,
                        )
                    nc.tensor.matmul(vps[:, 0, 0:128], ones_rowb, bv_sb,
                                     start=False, stop=False)
                    nc.tensor.matmul(vps[:, 0, 0:128],
                                     mr[:, tb * 128:(tb + 1) * 128], csv_sb,
                                     start=False, stop=True)
                    nc.vector.tensor_copy(vtm[:, ci * 4 + tb, :], vps[:, 0, 0:128])

            # ---- phase B: own-q LN1 + Q projection (2 chunks of 512) ----
            # qT layout [dq, hg, j, hh, q]: scores rhs [:, hg, j] is a fully
            # contiguous 512-column block (4 heads x 128 q).
            qT = s32p.tile([128, 4, 8, 4, 128], bf16, tag="s32", name="qT")
            for ch in range(2):
                xqc = strp.tile([128, HT, 512], bf16, tag="str16", name="xqc")
                nc.sync.dma_start(xqc, xq_d[ch])
                m_sb, rstd = _ln_chunk_stats(nc, workp, rowp, psB, psC, xqc,
                                             ones_col, eps_t)
                bcr = _bcast_row_bf16(nc, workp, psC, ones_rowf, rstd, "qbcr")
                mr = workp.tile([1, 512], bf16, tag="xq1k", bufs=2, name="qmr")
                nc.vector.tensor_mul(mr, m_sb, rstd)
                _scale_inplace(nc, xqc, bcr)
                for m in range(NH):
                    hg, hh = divmod(m, 4)
                    band = bandp.tile([128, HT, 128], bf16, tag="band4",
                                      bufs=3, name="band")
                    nc.sync.dma_start(band, wq_d[m])
                    qps = psA.tile([128, 2, 512], f32, tag="A", bufs=2, name="qps")
                    for kt in range(HT):
                        nc.tensor.matmul(qps[:, 0, :], band[:, kt, :],
                                         xqc[:, kt, :],
                                         start=(kt == 0), stop=False)
                    nc.tensor.matmul(qps[:, 0, :], csq_sb[:, m, :], mr,
                                     start=False, stop=True)
                    nc.scalar.activation(qT[:, hg, 4 * ch:4 * ch + 4, hh, :],
                                         qps[:, 0, :], AF.Identity,
                                         bias=bq_sb[:, m:m + 1])

            # ---- phase C: attention (scoresT, padded extent E=2j+2) ----
            # Two q-blocks' ladders are interleaved per head-group so the PE
            # fills the ACT-exp latency of one block with the other block's
            # matmuls.
            attnT = s32p.tile([128, NH, NQ], bf16, tag="s32", name="attnT")

            def _attn_step(hg, j, p0, kts, exacc, av_ps):
                """One 2-kt step of block j: two scores matmuls into a 2-bank
                psum tile, one exp over both, av accumulation on the PE and
                elementwise exp accumulation on the (otherwise idle) DVE —
                the softmax denominator then needs only ONE ones-matmul per
                block instead of one per k-tile."""
                E = len(kts)
                sc = psA.tile([128, 2, 512], f32, tag="A", bufs=2, name="sc")
                for dk in range(2):
                    kt = kts[p0 + dk]
                    masked = kt >= E - 2
                    nc.tensor.matmul(
                        sc[:, dk, :], kT_sb[:, kt * 128:(kt + 1) * 128],
                        qT[:, hg, j], start=True, stop=not masked,
                    )
                    if masked:
                        # mask add on the PE: sc += I^T @ mask
                        nc.tensor.matmul(sc[:, dk, :], id_bf,
                                         maskA if kt == E - 2 else maskB,
                                         start=False, stop=True)
                ex = workp.tile([128, 2, 512], bf16, tag="t2k", bufs=2,
                                name="ex")
                nc.scalar.activation(ex, sc, AF.Exp)
                if p0 == 0:
                    nc.vector.tensor_copy(exacc, ex[:, 0, :])
                else:
                    nc.vector.tensor_add(exacc, exacc, ex[:, 0, :])
                nc.vector.tensor_add(exacc, exacc, ex[:, 1, :])
                for dk in range(2):
                    idx = p0 + dk
                    kt = kts[idx]
                    nc.tensor.matmul(av_ps, vtm[:, kt, :], ex[:, dk, :],
                                     start=(idx == 0), stop=(idx == E - 1))

            def _attn_tail(hg, j, exacc, av_ps):
                den_ps = psB.tile([1, 512], f32, tag="B", bufs=2,
                                  name="den_ps")
                nc.tensor.matmul(den_ps, ones_col, exacc, start=True, stop=True)
                lnd = rowp.tile([1, 512], f32, tag="rows", bufs=2, name="lnd")
                nc.scalar.activation(lnd, den_ps, AF.Ln)
                nc.scalar.activation(lnd, lnd, AF.Exp, scale=-1.0)
                rec = lnd
                bcr_ps = psB.tile([128, 512], f32, tag="B", bufs=2,
                                  name="bcr_ps")
                nc.tensor.matmul(bcr_ps, ones_rowf, rec, start=True, stop=True)
                bcr_sb = workp.tile([128, 512], bf16, tag="bc1k", bufs=2,
                                    name="bcr_sb")
                nc.scalar.copy(bcr_sb, bcr_ps)
                nc.vector.tensor_mul(
                    attnT[:, hg * 4:(hg + 1) * 4, j * 128:(j + 1) * 128],
                    av_ps, bcr_sb,
                )

            for hg in range(4):
                for jp in range(4):
                    j0, j1 = 2 * jp, 2 * jp + 1
                    E0, E1 = 2 * j0 + 2, 2 * j1 + 2
                    kts0 = [E0 - 2, E0 - 1] + list(range(E0 - 2))
                    kts1 = [E1 - 2, E1 - 1] + list(range(E1 - 2))
                    ea0 = workp.tile([128, 512], bf16, tag="exac", bufs=2,
                                     name="ea0")
                    av0 = psC.tile([128, 512], f32, tag="C", bufs=2, name="av0")
                    ea1 = workp.tile([128, 512], bf16, tag="exac", bufs=2,
                                     name="ea1")
                    av1 = psC.tile([128, 512], f32, tag="C", bufs=2, name="av1")
                    for p0 in range(0, E1, 2):
                        if p0 < E0:
                            _attn_step(hg, j0, p0, kts0, ea0, av0)
                        elif p0 == E0:
                            _attn_tail(hg, j0, ea0, av0)
                        _attn_step(hg, j1, p0, kts1, ea1, av1)
                    _attn_tail(hg, j1, ea1, av1)

            # ---- phase D: out-proj + residual -> y ----
            # ch-outer: y[:, :, ch0] completes early so LN2+fc of chunk 0
            # overlap the second wo half.
            y = s32p.tile([128, HT, NQ], bf16, tag="s32", name="y")
            for ch in range(2):
                for ob in range(HT):
                    band = bandp.tile([128, HT, 128], bf16, tag="band4", bufs=3,
                                      name="band")
                    nc.sync.dma_start(band, wo_d[ob])
                    wps = psA.tile([128, 2, 512], f32, tag="A", bufs=2,
                                   name="wps")
                    for ht in range(HT):
                        nc.tensor.matmul(wps[:, 0, :], band[:, ht, :],
                                         attnT[:, ht, ch * 512:(ch + 1) * 512],
                                         start=(ht == 0), stop=(ht == HT - 1))
                    xqt = workp.tile([128, 512], bf16, tag="xq1k", bufs=2,
                                     name="xqt")
                    nc.sync.dma_start(xqt, xtq_d[ob, ch])
                    nc.vector.scalar_tensor_tensor(
                        out=y[:, ob, ch * 512:(ch + 1) * 512],
                        in0=wps[:, 0, :], scalar=bo_sb[:, ob:ob + 1],
                        in1=xqt, op0=ALU.add, op1=ALU.add,
                    )

            # ---- phase E: LN2 + MLP + residual -> out (per 512-token chunk)
            for ch in range(2):
                cols = slice(ch * 512, (ch + 1) * 512)
                sum_ps = psB.tile([1, 512], f32, tag="B", bufs=2, name="l2sum")
                sumsq_ps = psC.tile([1, 512], f32, tag="C", bufs=2, name="l2sq")
                for kt in range(HT):
                    nc.tensor.matmul(sum_ps, ones_col, y[:, kt, cols],
                                     start=(kt == 0), stop=(kt == HT - 1))
                for kt in range(HT):
                    sq = workp.tile([128, 512], bf16, tag="t2k", bufs=2,
                                    name="sq2")
                    nc.scalar.square(sq, y[:, kt, cols])
                    nc.tensor.matmul(sumsq_ps, ones_col, sq,
                                     start=(kt == 0), stop=(kt == HT - 1))
                m_sb, rstd = _ln_rows(nc, rowp, workp, psB, sum_ps, sumsq_ps, eps_t)
                bcm = _bcast_row_bf16(nc, workp, psC, ones_rowf, m_sb, "l2bcm")
                bcr = _bcast_row_bf16(nc, workp, psC, ones_rowf, rstd, "l2bcr")
                ln2s = s32p.tile([128, HT, 512], bf16, tag="s32", name="ln2s")
                for kt in range(HT):
                    nc.vector.tensor_sub(ln2s[:, kt, :], y[:, kt, cols], bcm)
                    nc.vector.tensor_mul(ln2s[:, kt, :], ln2s[:, kt, :], bcr)

                gT = bigp.tile([128, IT, 512], bf16, tag="big", name="gT")
                for mb in range(IT):
                    band = bandp.tile([128, HT, 128], bf16, tag="band4",
                                      bufs=3, name="band")
                    nc.sync.dma_start(band, wfc_d[mb])
                    fps = psA.tile([128, 2, 512], f32, tag="A", bufs=2,
                                   name="fps")
                    for kt in range(HT):
                        nc.tensor.matmul(fps[:, 0, :], band[:, kt, :],
                                         ln2s[:, kt, :],
                                         start=(kt == 0), stop=(kt == HT - 1))
                    nc.scalar.activation(gT[:, mb, :], fps[:, 0, :],
                                         AF.Gelu_apprx_tanh,
                                         bias=bfc_sb[:, mb:mb + 1])

                for ob in range(HT):
                    pband = strp.tile([128, IT, 128], bf16, tag="str16",
                                      name="pband")
                    nc.sync.dma_start(pband, wproj_d[ob])
                    pps = psA.tile([128, 2, 512], f32, tag="A", bufs=2,
                                   name="pps")
                    for mt in range(IT):
                        nc.tensor.matmul(pps[:, 0, :], pband[:, mt, :],
                                         gT[:, mt, :],
                                         start=(mt == 0), stop=(mt == IT - 1))
                    osb = workp.tile([128, 512], f32, tag="f2k", bufs=2,
                                     name="osb")
                    nc.vector.scalar_tensor_tensor(
                        out=osb, in0=pps[:, 0, :],
                        scalar=bproj_sb[:, ob:ob + 1],
                        in1=y[:, ob, cols], op0=ALU.add, op1=ALU.add,
                    )
                    nc.sync.dma_start(
                        out_d[ob * 128:(ob + 1) * 128, cols], osb
                    )
    _split_excess_waits(nc)
    return nc


_PROG = None


def _get_prog():
    global _PROG
    if _PROG is None:
        _PROG = _build_program()
    return _PROG


def _to_bf(a):
    return np.ascontiguousarray(a.astype(ml_dtypes.bfloat16))


def kernel(hidden_states, ln1_g, ln1_b, ln2_g, ln2_b, wq, bq, wkv, bkv,
           wo, bo, wfc, bfc, wproj, bproj):
    hs = np.asarray(hidden_states, np.float32)
    ln1_g = np.asarray(ln1_g, np.float32)
    ln1_b = np.asarray(ln1_b, np.float32)
    ln2_g = np.asarray(ln2_g, np.float32)
    ln2_b = np.asarray(ln2_b, np.float32)
    wq = np.asarray(wq, np.float32)
    wkv = np.asarray(wkv, np.float32)
    wo = np.asarray(wo, np.float32)
    wfc = np.asarray(wfc, np.float32)
    wproj = np.asarray(wproj, np.float32)

    # Fold LN gains into the following matmuls; fold qk scale into K.
    wq_e = ln1_g[:, None] * wq
    bq_e = np.asarray(bq, np.float32) + ln1_b @ wq
    wkv_e = ln1_g[:, None] * wkv
    bkv_e = np.asarray(bkv, np.float32) + ln1_b @ wkv
    scale = 1.0 / np.sqrt(D)
    wk_e = wkv_e[:, :D] * scale
    bk_e = bkv_e[:D] * scale
    wv_e = wkv_e[:, D:]
    bv_e = bkv_e[D:]
    wfc_e = ln2_g[:, None] * wfc
    bfc_e = np.asarray(bfc, np.float32) + ln2_b @ wfc

    # Host-packed weight layouts: [out-block, partition, k-tile, n] so each
    # band DMA is contiguous per partition line.
    wq_l = _to_bf(wq_e.reshape(HT, 128, NH, 128).transpose(2, 1, 0, 3))
    wk_l = _to_bf(wk_e.reshape(HT, 128, 128).transpose(1, 0, 2))
    wv_l = _to_bf(wv_e.reshape(HT, 128, 128).transpose(1, 0, 2))
    wo_l = _to_bf(wo.reshape(HT, 128, HT, 128).transpose(2, 1, 0, 3))
    wfc_l = _to_bf(wfc_e.reshape(HT, 128, IT, 128).transpose(2, 1, 0, 3))
    wproj_l = _to_bf(wproj.reshape(IT, 128, HT, 128).transpose(2, 1, 0, 3))

    # Negated column sums for the K=1 LN-mean-fold correction matmuls.
    csq_r = _to_bf(-wq_e.sum(axis=0).reshape(1, NH, 128))
    csk_r = _to_bf(-wk_e.sum(axis=0)[None, :])
    csv_r = _to_bf(-wv_e.sum(axis=0)[None, :])

    bq_r = np.ascontiguousarray(bq_e.reshape(NH, 128).T)
    bk_r = np.ascontiguousarray(bk_e[:, None])
    bv_r = _to_bf(bv_e[None, :])
    bo_r = np.ascontiguousarray(np.asarray(bo, np.float32).reshape(HT, 128).T)
    bfc_r = np.ascontiguousarray(bfc_e.reshape(IT, 128).T)
    bproj_r = np.ascontiguousarray(
        np.asarray(bproj, np.float32).reshape(HT, 128).T)

    # Causal masks for the two parity-dependent diagonal k-tiles.
    tri = np.where(np.arange(128)[None, :] >= np.arange(128)[:, None],
                   0.0, NEG).astype(np.float32)          # [k,q]
    tri4 = np.tile(tri, (1, 4))                          # [128, 512] (4 heads)
    zeros4 = np.zeros((128, 512), np.float32)
    neg4 = np.full((128, 512), NEG, np.float32)
    mask_h = [(_to_bf(tri4), _to_bf(neg4)),              # parity 0: (A, B)
              (_to_bf(zeros4), _to_bf(tri4))]            # parity 1: (A, B)

    in_maps = []
    gmaps = []
    for c in range(8):
        b, h = divmod(c, 2)
        gmap = [2 * j + h for j in range(8)]
        gmaps.append(gmap)
        xb = hs[b]                                        # [2048, 2048]
        xt_h = _to_bf(xb.reshape(NCH, 512, HT, 128).transpose(0, 3, 2, 1))
        xqb = xb.reshape(16, 128, H)[gmap].reshape(NQ, H)  # [1024, 2048]
        xq_h = _to_bf(xqb.reshape(2, 512, HT, 128).transpose(0, 3, 2, 1))
        xtq_h = _to_bf(xqb.reshape(2, 512, HT, 128).transpose(2, 0, 3, 1))
        mA, mB = mask_h[h]
        in_maps.append(dict(
            xt=xt_h, xq=xq_h, xtq=xtq_h,
            wq=wq_l, wk=wk_l, wv=wv_l, wo=wo_l, wfc=wfc_l, wproj=wproj_l,
            bq=bq_r, bk=bk_r, bv=bv_r, bo=bo_r, bfc=bfc_r, bproj=bproj_r,
            maskA=mA, maskB=mB, csq=csq_r, csk=csk_r, csv=csv_r,
        ))

    res = run_bass_kernel_spmd(_get_prog(), in_maps, core_ids=list(range(8)))
    kernel.last_result = res

    out = np.empty((B, S, H), np.float32)
    for c in range(8):
        b, h = divmod(c, 2)
        resT = np.asarray(res.results[c]["out"])          # [2048, 1024]
        blocks = resT.T.reshape(8, 128, H)                # local q-blocks
        for j, g in enumerate(gmaps[c]):
            out[b, g * 128:(g + 1) * 128, :] = blocks[j]
    return out


kernel.last_result = None


# revision 5
# speedup vs baseline: 1.0652x; 1.0112x over previous
"""GPTBigCode transformer block (MQA) on 8 trn2 NeuronCores — v2.

Sharding: data-parallel over batch (4) x parity-interleaved q-block split
(2) per batch element. Core c handles batch c//2 and q-blocks {2j + c%2}.
No collectives; K/V (single MQA head) recomputed per core.

v2 keeps ALL activations feature-on-partition ("T layout") end-to-end —
zero PE transposes. LayerNorm statistics are computed with ones-vector
matmul chains (partition-axis reduction on the tensor engine), per-token
scalars are broadcast back across partitions with K=1 matmuls. Attention
computes transposed scores (keys-on-partition) so softmax-denominators
come from ones-matmuls and probs feed attn@V directly. The softmax
normalization is applied as a per-column multiply on the attention
output. Causal masking of the parity-dependent diagonal zone uses two
per-core mask inputs so the compiled program is identical on all cores.

Weights are host-packed so every weight DMA is contiguous per partition
line; activations never round-trip through DRAM. Matmul inputs bf16;
accumulation, softmax and residual math f32 (residual stream bf16).
"""

import numpy as np
import ml_dtypes

# ---------------------------------------------------------------------------
# Workaround: this container's walrus build rejects >1 sync-wait on
# CTRL-class (Drain) instructions. Split the Tile tail-drain's waits into
# individual wait-carrying NOPs on the SP engine.
import bass_rust
from concourse.tile import TileContext
from concourse.vector_clock import ScopedClock


def _patched_drain_and_barrier(self, tick_clock, wait_clock):
    nc = self.nc
    drain_inst = nc.sync.drain()
    wait_clock.add_sem_waits(
        drain_inst.ins, ScopedClock({None: tick_clock.global_clock})
    )
    si = drain_inst.ins.sync_info
    waits = list(si.on_wait) if si and si.on_wait else []
    if len(waits) > 1:
        drain_inst.ins.sync_info = bass_rust.SyncInfo(
            on_wait=waits[:1],
            on_update=list(si.on_update) if si.on_update else [],
        )
        for w in waits[1:]:
            n = nc.sync.nop(nofuse=True, hint="split_drain_wait")
            n.ins.sync_info = bass_rust.SyncInfo(on_wait=[w], on_update=[])
    nc.all_engine_barrier()
    assert self.sems is not None
    popped = nc._tile_sem_poison_stack.pop()
    assert popped is self._sem_poison
    nc.clear_and_free_semaphores(list(self.sems.allocated().values()))
    nc.all_engine_barrier()


TileContext._drain_and_barrier = _patched_drain_and_barrier


def _split_excess_waits(nc, max_waits=1):
    """Rewrite every instruction carrying more than `max_waits` sem-waits:
    excess waits move onto same-engine NOPs inserted just before it."""
    all_bbs = [bb for fn in nc.m.functions for bb in fn.blocks]
    for bb in all_bbs:
        insts = list(bb.instructions)
        new_list = []
        changed = False
        for inst in insts:
            si = inst.sync_info
            waits = list(si.on_wait) if si and si.on_wait else []
            if len(waits) > max_waits:
                changed = True
                inst.sync_info = bass_rust.SyncInfo(
                    on_wait=waits[:max_waits],
                    on_update=list(si.on_update) if si.on_update else [],
                )
                for w in waits[max_waits:]:
                    nop_bi = nc.engines[inst.engine].nop(
                        nofuse=True, hint="wsplit"
                    )
                    nop = nop_bi.ins
                    cur = nc.cur_bb.bb
                    cl = list(cur.instructions)
                    assert cl and cl[-1].name == nop.name, "nop not appended last"
                    cur.instructions = cl[:-1]
                    nop.sync_info = bass_rust.SyncInfo(on_wait=[w], on_update=[])
                    new_list.append(nop)
            new_list.append(inst)
        if changed:
            bb.instructions = new_list
# ---------------------------------------------------------------------------

import concourse.bass as bass
import concourse.mybir as mybir
from concourse.bass_utils import run_bass_kernel_spmd
from concourse.masks import make_identity

f32 = mybir.dt.float32
bf16 = mybir.dt.bfloat16
AF = mybir.ActivationFunctionType
ALU = mybir.AluOpType

H = 2048
NH = 16
D = 128
INTER = 8192
S = 2048
B = 4
NQ = 1024          # query tokens per core
HT = H // 128      # 16
IT = INTER // 128  # 64
NCH = S // 512     # 4 full-seq chunks
EPS = 1e-5
NEG = -30000.0
INV_H = 1.0 / H


def _ln_rows(nc, rowp, workp2, psB, sum_ps, sumsq_ps, eps_t):
    """[1,512] psum sums -> (m_sb f32, rstd_sb f32) row tiles."""
    m_sb = rowp.tile([1, 512], f32, tag="rows", bufs=2, name="m_sb")
    nc.scalar.mul(m_sb, sum_ps, INV_H)
    v_sb = rowp.tile([1, 512], f32, tag="rows", bufs=2, name="v_sb")
    nc.scalar.mul(v_sb, sumsq_ps, INV_H)
    m2 = workp2.tile([128, 512], f32, tag="t2k", bufs=2, name="m2")
    nc.vector.tensor_mul(m2[0:1, :], m_sb, m_sb)
    nc.vector.tensor_sub(v_sb, v_sb, m2[0:1, :])
    # rstd = exp(-0.5*ln(var+eps)) — keeps the whole row path on ScalarE
    # (DVE reciprocal on a 1-partition row is ~3.3us serial); both steps
    # in place so the rows tag needs only 2 slots.
    nc.scalar.activation(v_sb, v_sb, AF.Ln, bias=eps_t)
    nc.scalar.activation(v_sb, v_sb, AF.Exp, scale=-0.5)
    return m_sb, v_sb


def _ln_chunk_stats(nc, workp, rowp, psB, psC, xc, ones_col, eps_t):
    """LN stats for one [128,16,512] bf16 chunk (raw x, T layout). sum and
    sumsq live in different psum tags so consecutive chunks' stats chains
    double-buffer instead of serializing on one tag pair."""
    sum_ps = psB.tile([1, 512], f32, tag="B", bufs=2, name="sum_ps")
    sumsq_ps = psC.tile([1, 512], f32, tag="C", bufs=2, name="sumsq_ps")
    for kt in range(HT):
        nc.tensor.matmul(sum_ps, ones_col, xc[:, kt, :],
                         start=(kt == 0), stop=(kt == HT - 1))
    for kt in range(HT):
        sq = workp.tile([128, 512], bf16, tag="t2k", bufs=2, name="sq")
        nc.scalar.square(sq, xc[:, kt, :])
        nc.tensor.matmul(sumsq_ps, ones_col, sq,
                         start=(kt == 0), stop=(kt == HT - 1))
    return _ln_rows(nc, rowp, workp, psB, sum_ps, sumsq_ps, eps_t)


def _bcast_row_bf16(nc, workp, psC, ones_rowf, row_sb, name):
    """[1,512] f32 row -> [128,512] bf16 sbuf broadcast tile."""
    bc_ps = psC.tile([128, 512], f32, tag="C", bufs=2, name=f"{name}_ps")
    nc.tensor.matmul(bc_ps, ones_rowf, row_sb, start=True, stop=True)
    bc_sb = workp.tile([128, 512], bf16, tag="bc1k", bufs=2, name=f"{name}_sb")
    nc.scalar.copy(bc_sb, bc_ps)
    return bc_sb


def _scale_inplace(nc, xc, bcr):
    """xc *= rstd (per column). The mean is folded into the following
    matmul chains as a K=1 rank-1 correction with -colsum(W)."""
    for kt in range(HT):
        nc.vector.tensor_mul(xc[:, kt, :], xc[:, kt, :], bcr)


def _build_program():
    nc = bass.Bass(trn_type="TRN2")

    xt_d = nc.dram_tensor("xt", [NCH, 128, HT, 512], bf16, kind="ExternalInput")
    xq_d = nc.dram_tensor("xq", [2, 128, HT, 512], bf16, kind="ExternalInput")
    xtq_d = nc.dram_tensor("xtq", [HT, 2, 128, 512], bf16, kind="ExternalInput")
    wq_d = nc.dram_tensor("wq", [NH, 128, HT, 128], bf16, kind="ExternalInput")
    wk_d = nc.dram_tensor("wk", [128, HT, 128], bf16, kind="ExternalInput")
    wv_d = nc.dram_tensor("wv", [128, HT, 128], bf16, kind="ExternalInput")
    wo_d = nc.dram_tensor("wo", [HT, 128, HT, 128], bf16, kind="ExternalInput")
    wfc_d = nc.dram_tensor("wfc", [IT, 128, HT, 128], bf16, kind="ExternalInput")
    wproj_d = nc.dram_tensor("wproj", [HT, 128, IT, 128], bf16, kind="ExternalInput")
    bq_d = nc.dram_tensor("bq", [128, NH], f32, kind="ExternalInput")
    bk_d = nc.dram_tensor("bk", [128, 1], f32, kind="ExternalInput")
    bv_d = nc.dram_tensor("bv", [1, 128], bf16, kind="ExternalInput")
    bo_d = nc.dram_tensor("bo", [128, HT], f32, kind="ExternalInput")
    bfc_d = nc.dram_tensor("bfc", [128, IT], f32, kind="ExternalInput")
    bproj_d = nc.dram_tensor("bproj", [128, HT], f32, kind="ExternalInput")
    maskA_d = nc.dram_tensor("maskA", [128, 512], bf16, kind="ExternalInput")
    maskB_d = nc.dram_tensor("maskB", [128, 512], bf16, kind="ExternalInput")
    csq_d = nc.dram_tensor("csq", [1, NH, 128], bf16, kind="ExternalInput")
    csk_d = nc.dram_tensor("csk", [1, 128], bf16, kind="ExternalInput")
    csv_d = nc.dram_tensor("csv", [1, 128], bf16, kind="ExternalInput")
    out_d = nc.dram_tensor("out", [H, NQ], f32, kind="ExternalOutput")

    with TileContext(nc) as tc:
        with (
            tc.tile_pool(name="const", bufs=1) as constp,
            tc.tile_pool(name="big", bufs=1) as bigp,
            tc.tile_pool(name="s32", bufs=2) as s32p,
            tc.tile_pool(name="str16", bufs=2) as strp,
            tc.tile_pool(name="band", bufs=3) as bandp,
            tc.tile_pool(name="work", bufs=2) as workp,
            tc.tile_pool(name="rows", bufs=4) as rowp,
            tc.tile_pool(name="psA", bufs=2, space="PSUM") as psA,
            tc.tile_pool(name="psB", bufs=2, space="PSUM") as psB,
            tc.tile_pool(name="psC", bufs=2, space="PSUM") as psC,
        ):
            # First input chunk DMA goes out before the constant loads so
            # compute can start as early as possible.
            xc0 = strp.tile([128, HT, 512], bf16, tag="str16", name="xc")
            nc.sync.dma_start(xc0[:, 0:8, :], xt_d[0][:, 0:8, :])
            nc.sync.dma_start(xc0[:, 8:16, :], xt_d[0][:, 8:16, :])

            # ---- constants ----
            ones_col = constp.tile([128, 1], bf16, name="ones_col")
            nc.vector.memset(ones_col, 1.0)
            ones_rowf = constp.tile([1, 128], f32, name="ones_rowf")
            nc.vector.memset(ones_rowf, 1.0)
            ones_rowb = constp.tile([1, 128], bf16, name="ones_rowb")
            nc.vector.memset(ones_rowb, 1.0)
            eps_t = constp.tile([1, 1], f32, name="eps_t")
            nc.vector.memset(eps_t, EPS)
            bq_sb = constp.tile([128, NH], f32, name="bq_sb")
            nc.sync.dma_start(bq_sb, bq_d[:, :])
            bk_sb = constp.tile([128, 1], f32, name="bk_sb")
            nc.sync.dma_start(bk_sb, bk_d[:, :])
            bv_sb = constp.tile([1, 128], bf16, name="bv_sb")
            nc.sync.dma_start(bv_sb, bv_d[:, :])
            bo_sb = constp.tile([128, HT], f32, name="bo_sb")
            nc.sync.dma_start(bo_sb, bo_d[:, :])
            bfc_sb = constp.tile([128, IT], f32, name="bfc_sb")
            nc.sync.dma_start(bfc_sb, bfc_d[:, :])
            bproj_sb = constp.tile([128, HT], f32, name="bproj_sb")
            nc.sync.dma_start(bproj_sb, bproj_d[:, :])
            maskA = constp.tile([128, 512], bf16, name="maskA")
            nc.sync.dma_start(maskA, maskA_d[:, :])
            maskB = constp.tile([128, 512], bf16, name="maskB")
            nc.sync.dma_start(maskB, maskB_d[:, :])
            csq_sb = constp.tile([1, NH, 128], bf16, name="csq_sb")
            nc.sync.dma_start(csq_sb, csq_d[:, :, :])
            csk_sb = constp.tile([1, 128], bf16, name="csk_sb")
            nc.sync.dma_start(csk_sb, csk_d[:, :])
            csv_sb = constp.tile([1, 128], bf16, name="csv_sb")
            nc.sync.dma_start(csv_sb, csv_d[:, :])
            id_bf = constp.tile([128, 128], bf16, name="id_bf")
            make_identity(nc, id_bf)
            kT_sb = constp.tile([128, S], bf16, name="kT_sb")
            vtm = constp.tile([128, HT, 128], bf16, name="vtm")

            # ---- phase A: full-seq LN1 + K/V, streamed in 512-token chunks
            for ci in range(NCH):
                if ci == 0:
                    xc = xc0
                else:
                    xc = strp.tile([128, HT, 512], bf16, tag="str16", name="xc")
                    nc.sync.dma_start(xc, xt_d[ci])
                wk_sb = bandp.tile([128, HT, 128], bf16, tag="band4", bufs=3,
                                   name="wk_sb")
                nc.sync.dma_start(wk_sb, wk_d[:, :, :])
                wv_sb = bandp.tile([128, HT, 128], bf16, tag="band4", bufs=3,
                                   name="wv_sb")
                nc.sync.dma_start(wv_sb, wv_d[:, :, :])
                m_sb, rstd = _ln_chunk_stats(nc, workp, rowp, psB, psC, xc,
                                             ones_col, eps_t)
                bcr = _bcast_row_bf16(nc, workp, psC, ones_rowf, rstd, "bcr")
                mr = workp.tile([1, 512], bf16, tag="xq1k", bufs=2, name="mr")
                nc.vector.tensor_mul(mr, m_sb, rstd)
                _scale_inplace(nc, xc, bcr)
                # K^T chunk: [dk=128, 512 tokens]
                kps = psA.tile([128, 2, 512], f32, tag="A", bufs=2, name="kps")
                for kt in range(HT):
                    nc.tensor.matmul(kps[:, 0, :], wk_sb[:, kt, :], xc[:, kt, :],
                                     start=(kt == 0), stop=False)
                nc.tensor.matmul(kps[:, 0, :], csk_sb, mr,
                                 start=False, stop=True)
                nc.scalar.activation(kT_sb[:, ci * 512:(ci + 1) * 512],
                                     kps[:, 0, :], AF.Identity, bias=bk_sb)
                # V token-major: 4 token-blocks
                for tb in range(4):
                    vps = psA.tile([128, 2, 512], f32, tag="A", bufs=2, name="vps")
                    for kt in range(HT):
                        nc.tensor.matmul(
                            vps[:, 0, 0:128],
                            xc[:, kt, tb * 128:(tb + 1) * 128],
                            wv_sb[:, kt, :], start=(kt == 0), stop=False,
                        )
                    nc.tensor.matmul(vps[:, 0, 0:128], ones_rowb, bv_sb,
                                     start=False, stop=False)
                    nc.tensor.matmul(vps[:, 0, 0:128],
                                     mr[:, tb * 128:(tb + 1) * 128], csv_sb,
                                     start=False, stop=True)
                    nc.vector.tensor_copy(vtm[:, ci * 4 + tb, :], vps[:, 0, 0:128])

            # ---- phase B: own-q LN1 + Q projection (2 chunks of 512) ----
            # qT layout [dq, hg, j, hh, q]: scores rhs [:, hg, j] is a fully
            # contiguous 512-column block (4 heads x 128 q).
            qT = s32p.tile([128, 4, 8, 4, 128], bf16, tag="s32", name="qT")
            for ch in range(2):
                xqc = strp.tile([128, HT, 512], bf16, tag="str16", name="xqc")
                nc.sync.dma_start(xqc, xq_d[ch])
                m_sb, rstd = _ln_chunk_stats(nc, workp, rowp, psB, psC, xqc,
                                             ones_col, eps_t)
                bcr = _bcast_row_bf16(nc, workp, psC, ones_rowf, rstd, "qbcr")
                mr = workp.tile([1, 512], bf16, tag="xq1k", bufs=2, name="qmr")
                nc.vector.tensor_mul(mr, m_sb, rstd)
                _scale_inplace(nc, xqc, bcr)
                for m in range(NH):
                    hg, hh = divmod(m, 4)
                    band = bandp.tile([128, HT, 128], bf16, tag="band4",
                                      bufs=3, name="band")
                    nc.sync.dma_start(band, wq_d[m])
                    qps = psA.tile([128, 2, 512], f32, tag="A", bufs=2, name="qps")
                    for kt in range(HT):
                        nc.tensor.matmul(qps[:, 0, :], band[:, kt, :],
                                         xqc[:, kt, :],
                                         start=(kt == 0), stop=False)
                    nc.tensor.matmul(qps[:, 0, :], csq_sb[:, m, :], mr,
                                     start=False, stop=True)
                    nc.scalar.activation(qT[:, hg, 4 * ch:4 * ch + 4, hh, :],
                                         qps[:, 0, :], AF.Identity,
                                         bias=bq_sb[:, m:m + 1])

            # ---- phase C: attention (scoresT, padded extent E=2j+2) ----
            # Two q-blocks' ladders are interleaved per head-group so the PE
            # fills the ACT-exp latency of one block with the other block's
            # matmuls.
            attnT = s32p.tile([128, NH, NQ], bf16, tag="s32", name="attnT")

            def _attn_step(hg, j, p0, kts, exacc, av_ps):
                """One 2-kt step of block j: two scores matmuls into a 2-bank
                psum tile, one exp over both, av accumulation on the PE and
                elementwise exp accumulation on the (otherwise idle) DVE —
                the softmax denominator then needs only ONE ones-matmul per
                block instead of one per k-tile."""
                E = len(kts)
                sc = psA.tile([128, 2, 512], f32, tag="A", bufs=2, name="sc")
                for dk in range(2):
                    kt = kts[p0 + dk]
                    masked = kt >= E - 2
                    nc.tensor.matmul(
                        sc[:, dk, :], kT_sb[:, kt * 128:(kt + 1) * 128],
                        qT[:, hg, j], start=True, stop=not masked,
                    )
                    if masked:
                        # mask add on the PE: sc += I^T @ mask
                        nc.tensor.matmul(sc[:, dk, :], id_bf,
                                         maskA if kt == E - 2 else maskB,
                                         start=False, stop=True)
                ex = workp.tile([128, 2, 512], bf16, tag="t2k", bufs=2,
                                name="ex")
                nc.scalar.activation(ex, sc, AF.Exp)
                if p0 == 0:
                    nc.vector.tensor_copy(exacc, ex[:, 0, :])
                else:
                    nc.vector.tensor_add(exacc, exacc, ex[:, 0, :])
                nc.vector.tensor_add(exacc, exacc, ex[:, 1, :])
                for dk in range(2):
                    idx = p0 + dk
                    kt = kts[idx]
                    nc.tensor.matmul(av_ps, vtm[:, kt, :], ex[:, dk, :],
                                     start=(idx == 0), stop=(idx == E - 1))

            def _attn_tail(hg, j, exacc, av_ps):
                den_ps = psB.tile([1, 512], f32, tag="B", bufs=2,
                                  name="den_ps")
                nc.tensor.matmul(den_ps, ones_col, exacc, start=True, stop=True)
                lnd = rowp.tile([1, 512], f32, tag="rows", bufs=2, name="lnd")
                nc.scalar.activation(lnd, den_ps, AF.Ln)
                nc.scalar.activation(lnd, lnd, AF.Exp, scale=-1.0)
                rec = lnd
                bcr_ps = psB.tile([128, 512], f32, tag="B", bufs=2,
                                  name="bcr_ps")
                nc.tensor.matmul(bcr_ps, ones_rowf, rec, start=True, stop=True)
                bcr_sb = workp.tile([128, 512], bf16, tag="bc1k", bufs=2,
                                    name="bcr_sb")
                nc.scalar.copy(bcr_sb, bcr_ps)
                nc.vector.tensor_mul(
                    attnT[:, hg * 4:(hg + 1) * 4, j * 128:(j + 1) * 128],
                    av_ps, bcr_sb,
                )

            for hg in range(4):
                for jp in range(4):
                    j0, j1 = 2 * jp, 2 * jp + 1
                    E0, E1 = 2 * j0 + 2, 2 * j1 + 2
                    kts0 = [E0 - 2, E0 - 1] + list(range(E0 - 2))
                    kts1 = [E1 - 2, E1 - 1] + list(range(E1 - 2))
                    ea0 = workp.tile([128, 512], bf16, tag="exac", bufs=2,
                                     name="ea0")
                    av0 = psC.tile([128, 512], f32, tag="C", bufs=2, name="av0")
                    ea1 = workp.tile([128, 512], bf16, tag="exac", bufs=2,
                                     name="ea1")
                    av1 = psC.tile([128, 512], f32, tag="C", bufs=2, name="av1")
                    for p0 in range(0, E1, 2):
                        if p0 < E0:
                            _attn_step(hg, j0, p0, kts0, ea0, av0)
                        elif p0 == E0:
                            _attn_tail(hg, j0, ea0, av0)
                        _attn_step(hg, j1, p0, kts1, ea1, av1)
                    _attn_tail(hg, j1, ea1, av1)

            # ---- phase D: out-proj + residual -> y ----
            # ch-outer: y[:, :, ch0] completes early so LN2+fc of chunk 0
            # overlap the second wo half.
            y = s32p.tile([128, HT, NQ], bf16, tag="s32", name="y")
            for ch in range(2):
                for ob in range(HT):
                    band = bandp.tile([128, HT, 128], bf16, tag="band4", bufs=3,
                                      name="band")
                    nc.sync.dma_start(band, wo_d[ob])
                    wps = psA.tile([128, 2, 512], f32, tag="A", bufs=2,
                                   name="wps")
                    for ht in range(HT):
                        nc.tensor.matmul(wps[:, 0, :], band[:, ht, :],
                                         attnT[:, ht, ch * 512:(ch + 1) * 512],
                                         start=(ht == 0), stop=(ht == HT - 1))
                    xqt = workp.tile([128, 512], bf16, tag="xq1k", bufs=2,
                                     name="xqt")
                    nc.sync.dma_start(xqt, xtq_d[ob, ch])
                    nc.vector.scalar_tensor_tensor(
                        out=y[:, ob, ch * 512:(ch + 1) * 512],
                        in0=wps[:, 0, :], scalar=bo_sb[:, ob:ob + 1],
                        in1=xqt, op0=ALU.add, op1=ALU.add,
                    )

            # ---- phase E: LN2 + MLP + residual -> out (per 512-token chunk)
            for ch in range(2):
                cols = slice(ch * 512, (ch + 1) * 512)
                # LN2 stats: elementwise accumulate on the DVE (idle in
                # this PE-dense region), then a single ones-matmul each.
                yacc = workp.tile([128, 512], bf16, tag="exac", bufs=2,
                                  name="yacc")
                nc.vector.tensor_copy(yacc, y[:, 0, cols])
                sqacc = workp.tile([128, 512], bf16, tag="exac", bufs=2,
                                   name="sqacc")
                nc.scalar.square(sqacc, y[:, 0, cols])
                for kt in range(1, HT):
                    nc.vector.tensor_add(yacc, yacc, y[:, kt, cols])
                    sq = workp.tile([128, 512], bf16, tag="t2k", bufs=2,
                                    name="sq2")
                    nc.scalar.square(sq, y[:, kt, cols])
                    nc.vector.tensor_add(sqacc, sqacc, sq)
                sum_ps = psB.tile([1, 512], f32, tag="B", bufs=2, name="l2sum")
                nc.tensor.matmul(sum_ps, ones_col, yacc, start=True, stop=True)
                sumsq_ps = psC.tile([1, 512], f32, tag="C", bufs=2, name="l2sq")
                nc.tensor.matmul(sumsq_ps, ones_col, sqacc, start=True, stop=True)
                m_sb, rstd = _ln_rows(nc, rowp, workp, psB, sum_ps, sumsq_ps, eps_t)
                bcm = _bcast_row_bf16(nc, workp, psC, ones_rowf, m_sb, "l2bcm")
                bcr = _bcast_row_bf16(nc, workp, psC, ones_rowf, rstd, "l2bcr")
                ln2s = s32p.tile([128, HT, 512], bf16, tag="s32", name="ln2s")
                for kt in range(HT):
                    nc.vector.tensor_sub(ln2s[:, kt, :], y[:, kt, cols], bcm)
                    nc.vector.tensor_mul(ln2s[:, kt, :], ln2s[:, kt, :], bcr)

                gT = bigp.tile([128, IT, 512], bf16, tag="big", name="gT")
                for mb in range(IT):
                    band = bandp.tile([128, HT, 128], bf16, tag="band4",
                                      bufs=3, name="band")
                    nc.sync.dma_start(band, wfc_d[mb])
                    fps = psA.tile([128, 2, 512], f32, tag="A", bufs=2,
                                   name="fps")
                    for kt in range(HT):
                        nc.tensor.matmul(fps[:, 0, :], band[:, kt, :],
                                         ln2s[:, kt, :],
                                         start=(kt == 0), stop=(kt == HT - 1))
                    nc.scalar.activation(gT[:, mb, :], fps[:, 0, :],
                                         AF.Gelu_apprx_tanh,
                                         bias=bfc_sb[:, mb:mb + 1])

                for ob in range(HT):
                    pband = strp.tile([128, IT, 128], bf16, tag="str16",
                                      name="pband")
                    nc.sync.dma_start(pband, wproj_d[ob])
                    pps = psA.tile([128, 2, 512], f32, tag="A", bufs=2,
                                   name="pps")
                    for mt in range(IT):
                        nc.tensor.matmul(pps[:, 0, :], pband[:, mt, :],
                                         gT[:, mt, :],
                                         start=(mt == 0), stop=(mt == IT - 1))
                    osb = workp.tile([128, 512], f32, tag="f2k", bufs=2,
                                     name="osb")
                    nc.vector.scalar_tensor_tensor(
                        out=osb, in0=pps[:, 0, :],
                        scalar=bproj_sb[:, ob:ob + 1],
                        in1=y[:, ob, cols], op0=ALU.add, op1=ALU.add,
                    )
                    nc.sync.dma_start(
                        out_d[ob * 128:(ob + 1) * 128, cols], osb
                    )
    _split_excess_waits(nc)
    return nc


_PROG = None


def _get_prog():
    global _PROG
    if _PROG is None:
        _PROG = _build_program()
    return _PROG


def _to_bf(a):
    return np.ascontiguousarray(a.astype(ml_dtypes.bfloat16))


def kernel(hidden_states, ln1_g, ln1_b, ln2_g, ln2_b, wq, bq, wkv, bkv,
           wo, bo, wfc, bfc, wproj, bproj):
    hs = np.asarray(hidden_states, np.float32)
    ln1_g = np.asarray(ln1_g, np.float32)
    ln1_b = np.asarray(ln1_b, np.float32)
    ln2_g = np.asarray(ln2_g, np.float32)
    ln2_b = np.asarray(ln2_b, np.float32)
    wq = np.asarray(wq, np.float32)
    wkv = np.asarray(wkv, np.float32)
    wo = np.asarray(wo, np.float32)
    wfc = np.asarray(wfc, np.float32)
    wproj = np.asarray(wproj, np.float32)

    # Fold LN gains into the following matmuls; fold qk scale into K.
    wq_e = ln1_g[:, None] * wq
    bq_e = np.asarray(bq, np.float32) + ln1_b @ wq
    wkv_e = ln1_g[:, None] * wkv
    bkv_e = np.asarray(bkv, np.float32) + ln1_b @ wkv
    scale = 1.0 / np.sqrt(D)
    wk_e = wkv_e[:, :D] * scale
    bk_e = bkv_e[:D] * scale
    wv_e = wkv_e[:, D:]
    bv_e = bkv_e[D:]
    wfc_e = ln2_g[:, None] * wfc
    bfc_e = np.asarray(bfc, np.float32) + ln2_b @ wfc

    # Host-packed weight layouts: [out-block, partition, k-tile, n] so each
    # band DMA is contiguous per partition line.
    wq_l = _to_bf(wq_e.reshape(HT, 128, NH, 128).transpose(2, 1, 0, 3))
    wk_l = _to_bf(wk_e.reshape(HT, 128, 128).transpose(1, 0, 2))
    wv_l = _to_bf(wv_e.reshape(HT, 128, 128).transpose(1, 0, 2))
    wo_l = _to_bf(wo.reshape(HT, 128, HT, 128).transpose(2, 1, 0, 3))
    wfc_l = _to_bf(wfc_e.reshape(HT, 128, IT, 128).transpose(2, 1, 0, 3))
    wproj_l = _to_bf(wproj.reshape(IT, 128, HT, 128).transpose(2, 1, 0, 3))

    # Negated column sums for the K=1 LN-mean-fold correction matmuls.
    csq_r = _to_bf(-wq_e.sum(axis=0).reshape(1, NH, 128))
    csk_r = _to_bf(-wk_e.sum(axis=0)[None, :])
    csv_r = _to_bf(-wv_e.sum(axis=0)[None, :])

    bq_r = np.ascontiguousarray(bq_e.reshape(NH, 128).T)
    bk_r = np.ascontiguousarray(bk_e[:, None])
    bv_r = _to_bf(bv_e[None, :])
    bo_r = np.ascontiguousarray(np.asarray(bo, np.float32).reshape(HT, 128).T)
    bfc_r = np.ascontiguousarray(bfc_e.reshape(IT, 128).T)
    bproj_r = np.ascontiguousarray(
        np.asarray(bproj, np.float32).reshape(HT, 128).T)

    # Causal masks for the two parity-dependent diagonal k-tiles.
    tri = np.where(np.arange(128)[None, :] >= np.arange(128)[:, None],
                   0.0, NEG).astype(np.float32)          # [k,q]
    tri4 = np.tile(tri, (1, 4))                          # [128, 512] (4 heads)
    zeros4 = np.zeros((128, 512), np.float32)
    neg4 = np.full((128, 512), NEG, np.float32)
    mask_h = [(_to_bf(tri4), _to_bf(neg4)),              # parity 0: (A, B)
              (_to_bf(zeros4), _to_bf(tri4))]            # parity 1: (A, B)

    in_maps = []
    gmaps = []
    for c in range(8):
        b, h = divmod(c, 2)
        gmap = [2 * j + h for j in range(8)]
        gmaps.append(gmap)
        xb = hs[b]                                        # [2048, 2048]
        xt_h = _to_bf(xb.reshape(NCH, 512, HT, 128).transpose(0, 3, 2, 1))
        xqb = xb.reshape(16, 128, H)[gmap].reshape(NQ, H)  # [1024, 2048]
        xq_h = _to_bf(xqb.reshape(2, 512, HT, 128).transpose(0, 3, 2, 1))
        xtq_h = _to_bf(xqb.reshape(2, 512, HT, 128).transpose(2, 0, 3, 1))
        mA, mB = mask_h[h]
        in_maps.append(dict(
            xt=xt_h, xq=xq_h, xtq=xtq_h,
            wq=wq_l, wk=wk_l, wv=wv_l, wo=wo_l, wfc=wfc_l, wproj=wproj_l,
            bq=bq_r, bk=bk_r, bv=bv_r, bo=bo_r, bfc=bfc_r, bproj=bproj_r,
            maskA=mA, maskB=mB, csq=csq_r, csk=csk_r, csv=csv_r,
        ))

    res = run_bass_kernel_spmd(_get_prog(), in_maps, core_ids=list(range(8)))
    kernel.last_result = res

    out = np.empty((B, S, H), np.float32)
    for c in range(8):
        b, h = divmod(c, 2)
        resT = np.asarray(res.results[c]["out"])          # [2048, 1024]
        blocks = resT.T.reshape(8, 128, H)                # local q-blocks
        for j, g in enumerate(gmaps[c]):
            out[b, g * 128:(g + 1) * 128, :] = blocks[j]
    return out


kernel.last_result = None
